# revision 47
# baseline (speedup 1.0000x reference)
"""Trainium2 Bass kernel for a 2-hidden-layer LIF spiking network.

Math (per timestep t, per layer):
    v = 0.9*y + cur ;  spike s = (v >= 1) ;  y = v*(1-s) = v*u  with u = (v < 1)
Layer currents:
    cur1 = x_t @ W_ih            (x binary, precomputable for ALL t)
    cur2 = s1 @ W_hh = colsum(W_hh) - u1 @ W_hh
    cur3 = s2 @ W_ho = colsum(W_ho) - u2 @ W_ho
Output: rate = mean_t s_out = 1 - sum_t(u_out)/T

Key restructurings:
  * Layer 1's recurrence does not depend on layer 2, so all three matmuls are
    batched over the full (T*B) column space; only the cheap elementwise LIF
    scans are sequential in t.
  * Weights are quantized to 24-bit fixed point (step 2^-24) and decomposed
    into NP=5 exact signed base-32 digit planes stored in fp8 e4m3 (digits in
    [-16,15] times power-of-2 scales are exact in e4m3). Pairs of digit
    planes feed fp8 DoubleRow matmuls (2 stationary planes per instruction at
    0.5 cycles/row), so full 24-bit weight precision streams at 1.25
    cycles/row: planes 0,1 pair within a k-chunk against a moving operand
    carrying 2^-15; plane 2 pairs across ADJACENT k-chunks (also vs 2^-15);
    planes 3,4 pair within a k-chunk against a SECOND moving copy carrying
    1.0 (e4m3's narrow exponent range cannot span 25 bits against a single
    moving scale, so the high planes are stored 2^15 lower and the moving
    side supplies the 2^15 back). Every product is exact in fp32 PSUM and
    all planes accumulate into ONE PSUM group -- no extra combine ops.
  * The moving spike-complement tiles are stored once per scale: u1 at 2^-15
    stays resident in SBUF for ALL timesteps; the {0,1} copy is staged per
    superblock by one cheap DVE op (off the critical scan chain).

Sharding: data-parallel over batch (256/8 = 32 rows per core), weights
replicated, no cross-core communication.

Per-core schedule:
  Phase A (W_ih digit planes resident, 10.5MB, loaded progressively per
    output chunk; blocks 0+1 are emitted interleaved per m-chunk so mm1
    rides the weight DMA): mm1 over blocks of 5 steps, LIF1 scan fused per
    block (v computed in-place over cur1), u1 written straight into its
    resident SBUF tile (never spilled).
  Phase B (W_hh digit planes streamed from DRAM per 128-col output chunk,
    triple-buffered; the first half-chunk prefetched during phase A into a
    disjoint tile so the phase seam has no DMA stall): superblocks of 20
    steps; mm2 -> cur2 with colsum correction fused into the PSUM->SBUF
    Identity-activation copy (scale=-1, bias=colsum), LIF2 scan (u2 written
    into a retired ring slot of the u1 tile), mm3 transposed (moving side =
    W_ho planes so its cost scales with the 10-wide output, then a PE
    transpose restores [10, cols] via an identity matmul), output LIF scan,
    final rate. mm3 for superblock s is emitted mid-way through superblock
    s+1's mm2 so the PE never waits on the DVE scan.
"""

import numpy as np

# ---- problem constants (hardcoded; kernel.py must be self-contained) ----
BATCH = 256
INPUT_DIM = 1024
HIDDEN_DIM = 2048
OUTPUT_DIM = 10
T = 100
NCORES = 8
BLOC = BATCH // NCORES          # 32 batch rows per core
TB = 5                          # timesteps per phase-A block
NBLK = T // TB                  # 20 blocks
CA = TB * BLOC                  # 160 columns per phase-A block
C = 320                         # columns per phase-B matmul chunk
COLS = T * BLOC                 # 3200 total columns
# phase-B superblocks (col0, ncols); u2 for super s overwrites u1's own
# slot s -- by the time the LIF2 scan emits u2(s), mm2(s) has consumed
# every u1 column in that slot (the scan is already ordered after mm2(s)
# through cur2), so no spare slot is needed.
SCMAX = 640
SUPERS = [(0, 640), (640, 640), (1280, 640), (1920, 640), (2560, 640)]
USLOT = [0, 1, 2, 3, 4]
NSLOT = 5
KI = INPUT_DIM // 128           # 8 k-chunks for mm1
KH = HIDDEN_DIM // 128          # 16 k-chunks (and m-chunks) for mm2
DECAY = 0.9
THRESH = 1.0
TH_NUDGE = 0.0                  # tie-break re-roll knob (harmless ~1e-6 scale)

# base-32 e4m3 digit planes. W_ih gets 5 planes (24-bit fixed point): the
# input layer is the flip-sensitive one (its quantization error enters v1
# directly every step). W_hh gets 4 planes (20-bit): measured flip count
# stays in the same singleton class as 24-bit, and mm2 dominates PE time.
NP_IH = 5
KBITS_IH = 24
# plane i contributes d_i * 32^i * 2^-24; planes 0-2 ride moving 2^-15,
# planes 3,4 ride moving 1.0 (stored 2^15 lower)
PSCALE_IH = [2.0 ** -9, 2.0 ** -4, 2.0 ** 1, 2.0 ** -9, 2.0 ** -4]
NP_HH = 4
KBITS_HH = 20
# plane i contributes d_i * 32^i * 2^-20; planes 0,1 ride moving 2^-15,
# planes 2,3 ride moving 2^-5 (stored 2^10 lower)
PSCALE_HH = [2.0 ** -5, 2.0 ** 0, 2.0 ** -5, 2.0 ** 0]
UHI_VAL = 2.0 ** 10             # u_hi = u_lo * 2^10 = {0, 2^-5}
MOV = 2.0 ** -15                # lo moving value (e5m2 subnormal, exact)
NIH = NP_IH * KI                # 40 plane-items per wih m-chunk
NHH = NP_HH * KH                # 64 plane-items per whh m2-chunk

# mm3 weight planes stay base-16 e5m2 (cost is negligible at 10-wide out)
ND6 = 6
KB23 = 23
DMAX16 = 7 * (16 ** ND6 - 1) // 15

_BUILT = None


def _half_items5(kc2, h):
    """5-plane item order within one half (kc2 k-chunks starting at h*kc2):
    A-pairs (planes 0,1 per k), C-pairs (plane 2 of adjacent k), B-pairs
    (planes 3,4 per k). Returns [(k, plane), ...], DR-pair-adjacent."""
    k0 = h * kc2
    items = []
    for k in range(k0, k0 + kc2):
        items += [(k, 0), (k, 1)]
    for k in range(k0, k0 + kc2):
        items.append((k, 2))
    for k in range(k0, k0 + kc2):
        items += [(k, 3), (k, 4)]
    return items


def _half_items4(kc2, h):
    """4-plane item order: A-pairs (planes 0,1 per k) then B-pairs (planes
    2,3 per k); no cross-k pair needed."""
    k0 = h * kc2
    items = []
    for k in range(k0, k0 + kc2):
        items += [(k, 0), (k, 1)]
    for k in range(k0, k0 + kc2):
        items += [(k, 2), (k, 3)]
    return items


def _build():
    """Trace + compile the Bass program once."""
    from contextlib import ExitStack

    import concourse.bacc as bacc
    import concourse.tile as tile
    from concourse import mybir
    from concourse.alu_op_type import AluOpType as op

    f32 = mybir.dt.float32
    e5 = mybir.dt.float8e5
    e4 = mybir.dt.float8e4
    DR = mybir.MatmulPerfMode.DoubleRow
    ident = mybir.ActivationFunctionType.Identity
    TH = THRESH + TH_NUDGE

    nc = bacc.Bacc("TRN2", target_bir_lowering=False, debug=False,
                   num_devices=NCORES)

    # x at both moving scales packed in one tensor (lo rows then hi rows):
    # [2*input_dim, t*b] t-major columns -- one DMA covers both scales
    x_d = nc.dram_tensor("xall", [2 * INPUT_DIM, COLS], e5,
                         kind="ExternalInput").ap()
    xlo_d = x_d[0:INPUT_DIM, :]
    xhi_d = x_d[INPUT_DIM:2 * INPUT_DIM, :]
    # wih planes: row (m*128+p) holds that partition's NIH plane-items
    wih_d = nc.dram_tensor("wihd", [KH * 128, NIH * 128], e4,
                           kind="ExternalInput").ap()
    # whh planes: row (m2*128+p) holds NHH plane-items (two k-halves)
    whh_d = nc.dram_tensor("whhd", [KH * 128, NHH * 128], e4,
                           kind="ExternalInput").ap()
    # who planes padded to 16 cols: [kt*128, dig*16] (e5m2 base-16)
    who_d = nc.dram_tensor("whod", [KH * 128, ND6 * 16], e5,
                           kind="ExternalInput").ap()
    cs_hh_d = nc.dram_tensor("cs_hh", [128, KH], f32, kind="ExternalInput").ap()
    cs_ho_d = nc.dram_tensor("cs_ho", [OUTPUT_DIM, 1], f32,
                             kind="ExternalInput").ap()
    id_d = nc.dram_tensor("ident", [128, 128], f32, kind="ExternalInput").ap()
    out_d = nc.dram_tensor("out", [OUTPUT_DIM, BLOC], f32,
                           kind="ExternalOutput").ap()

    with tile.TileContext(nc) as tc, ExitStack() as ctx:
        # spike complements {0, 2^-15}, resident across both phases
        # [p, kt, col]: cols 0..3200 hold u1; 6 ring slots of 640 also serve
        # as u2 staging (a slot is reused once mm2 has consumed its u1 cols)
        u1_pool = ctx.enter_context(tc.tile_pool(name="u1", bufs=1))
        u1 = u1_pool.tile([128, KH * NSLOT * SCMAX], e5, tag="u1")
        u1_3 = u1[:].rearrange("p (k c) -> p k c", k=KH)

        # {0,1} copies of u1, staged per superblock for the hi-plane pairs
        # (two 320-col halves per superblock, two superblocks in flight)
        uhi_pool = ctx.enter_context(tc.tile_pool(name="uhi", bufs=4))

        UHW = SCMAX // 2

        def stage_uhi(c0, cn):
            """Stage {0,1} copies of u1 cols [c0, c0+cn) as 320-col half
            tiles (halves the pool footprint vs full-superblock tiles)."""
            halves = []
            for hc0 in range(0, cn, UHW):
                w = min(UHW, cn - hc0)
                uh = uhi_pool.tile([128, KH * UHW], e5, tag="uhi",
                                   name=f"uhi_{c0 + hc0}")
                uh_3 = uh[:].rearrange("p (k c) -> p k c", k=KH)
                nc.vector.tensor_scalar(uh_3[:, :, 0:w],
                                        u1_3[:, :, c0 + hc0:c0 + hc0 + w],
                                        UHI_VAL, None, op.mult)
                halves.append(uh_3)
            return halves

        # first whh half-chunk (k 0-7 of m2=0), prefetched into space
        # disjoint from phase A's pools so its DMA runs during phase A
        wpre_pool = ctx.enter_context(tc.tile_pool(name="wpre", bufs=1))
        # phase-B PSUM pools live at the outer scope so their addresses are
        # DISJOINT from phase A's psA pool: otherwise mm2's first PSUM
        # writes WAR-wait on phase A's last PSUM->SBUF copies draining
        psB = ctx.enter_context(tc.tile_pool(name="psB", bufs=4,
                                             space="PSUM"))
        wst_pre = wpre_pool.tile([128, (NHH // 2) * 128], e4, tag="wpre")

        def emit_planes(ps, w_3, kc2, h, mov_lo, mov_hi, c0, cn, first, last,
                        c0h=None, parts="all"):
            """One half's 2.5*kc2 DoubleRow matmuls into PSUM group `ps`.
            w_3: stationary tile viewed [p, item, 128] (items for half h at
            item offset h*5*kc2). mov_lo/mov_hi: [p, k, c] moving tiles;
            c0h: column origin within mov_hi (defaults to c0). parts
            selects the lo-moving pairs ("ac"), hi-moving pairs ("b"), or
            everything ("all") so callers can front-load the work that only
            needs mov_lo while mov_hi's DMA is still in flight."""
            if c0h is None:
                c0h = c0
            base = h * 5 * kc2
            k0 = h * kc2
            idx = 0
            if parts in ("all", "ac"):
                for k in range(k0, k0 + kc2):    # A-pairs (planes 0,1)
                    nc.tensor.matmul(
                        ps, w_3[:, base + idx:base + idx + 2, :],
                        mov_lo[:, k, c0:c0 + cn].unsqueeze(1)
                        .broadcast_to([128, 2, cn]),
                        start=(first and idx == 0), stop=False, perf_mode=DR)
                    idx += 2
                for kk in range(kc2 // 2):       # C-pairs (plane 2, k/k+1)
                    nc.tensor.matmul(
                        ps, w_3[:, base + idx:base + idx + 2, :],
                        mov_lo[:, k0 + 2 * kk:k0 + 2 * kk + 2, c0:c0 + cn],
                        start=False, stop=False, perf_mode=DR)
                    idx += 2
            else:
                idx = 3 * kc2
            if parts in ("all", "b"):
                for k in range(k0, k0 + kc2):    # B-pairs (planes 3,4)
                    nc.tensor.matmul(
                        ps, w_3[:, base + idx:base + idx + 2, :],
                        mov_hi[:, k, c0h:c0h + cn].unsqueeze(1)
                        .broadcast_to([128, 2, cn]),
                        start=False, stop=(last and idx == 5 * kc2 - 2),
                        perf_mode=DR)
                    idx += 2

        def emit_planes4(ps, w_3, kc2, h, mov_lo, mov_hi, c0, cn, first,
                         last, c0h):
            """4-plane variant: 2*kc2 DoubleRow matmuls (A-pairs vs mov_lo,
            B-pairs vs mov_hi), items at offset h*4*kc2."""
            base = h * 4 * kc2
            k0 = h * kc2
            idx = 0
            for k in range(k0, k0 + kc2):        # A-pairs (planes 0,1)
                nc.tensor.matmul(
                    ps, w_3[:, base + idx:base + idx + 2, :],
                    mov_lo[:, k, c0:c0 + cn].unsqueeze(1)
                    .broadcast_to([128, 2, cn]),
                    start=(first and idx == 0), stop=False, perf_mode=DR)
                idx += 2
            for k in range(k0, k0 + kc2):        # B-pairs (planes 2,3)
                nc.tensor.matmul(
                    ps, w_3[:, base + idx:base + idx + 2, :],
                    mov_hi[:, k, c0h:c0h + cn].unsqueeze(1)
                    .broadcast_to([128, 2, cn]),
                    start=False, stop=(last and idx == 4 * kc2 - 2),
                    perf_mode=DR)
                idx += 2

        # ---------------- Phase A: mm1 + LIF1 scan ----------------
        with tc.tile_pool(name="wih", bufs=1) as wih_pool, \
             tc.tile_pool(name="xin", bufs=2) as x_pool, \
             tc.tile_pool(name="cur1", bufs=3) as cur1_pool, \
             tc.tile_pool(name="st1", bufs=1) as st1_pool, \
             tc.tile_pool(name="psA", bufs=4, space="PSUM") as psA:

            def load_x(xt, c0, ncols):
                # one DMA covers both scales (lo then hi in the free dim)
                xt_4 = xt[:].rearrange("p (s k c) -> p s k c", s=2, k=KI)
                nc.sync.dma_start(
                    xt_4,
                    x_d[:, c0:c0 + ncols].rearrange("(s k p) c -> p s k c",
                                                    p=128, s=2))
                return xt_4[:, 0], xt_4[:, 1]

            # wih digit planes, per m-chunk (progressive: mm1 m-chunk can
            # start as soon as its planes land); the m=0 chunk issues first,
            # then the two head x blocks, then the rest of the chunks
            def load_wih(m):
                w = wih_pool.tile([128, NIH * 128], e4, tag=f"wih_{m}")
                nc.sync.dma_start(w[:], wih_d[m * 128:(m + 1) * 128, :])
                return w

            # first wih half-load, then head-x lo, then the second half,
            # then head-x hi: the first A-pair matmuls start after just two
            # small DMAs instead of the whole m=0 chunk + both x tensors
            w0 = wih_pool.tile([128, NIH * 128], e4, tag="wih_0")
            halfb = (NIH // 2) * 128
            nc.sync.dma_start(w0[:, 0:halfb], wih_d[0:128, 0:halfb])
            x_head = x_pool.tile([128, 2 * KI * 3 * CA], e5, tag="xh",
                                 bufs=1)
            xh_4 = x_head[:].rearrange("p (s k c) -> p s k c", s=2, k=KI)
            nc.sync.dma_start(
                xh_4[:, 0],
                xlo_d[:, 0:3 * CA].rearrange("(k p) c -> p k c", p=128))
            nc.sync.dma_start(w0[:, halfb:], wih_d[0:128, halfb:])
            nc.sync.dma_start(
                xh_4[:, 1],
                xhi_d[:, 0:3 * CA].rearrange("(k p) c -> p k c", p=128))
            xhl_3, xhh_3 = xh_4[:, 0], xh_4[:, 1]
            wih_sb = [w0]
            for m in range(1, KH):
                wih_sb.append(load_wih(m))

            y1 = st1_pool.tile([128, KH * BLOC], f32, tag="y1")
            nc.vector.memset(y1[:], 0.0)
            y1_3 = y1[:].rearrange("p (m b) -> p m b", m=KH)

            def mm1_block(xl_3, xh_3, c0, cur1, m):
                ps = psA.tile([128, CA], f32, tag="psA")
                w_3 = wih_sb[m][:].rearrange("p (i f) -> p i f", i=NIH)
                # lo-moving pairs of both halves first: the x_hi DMA (the
                # later half of each x load) is never on the critical path
                for h in range(2):
                    emit_planes(ps[:], w_3, KI // 2, h, xl_3, xh_3,
                                c0, CA, first=(h == 0), last=False,
                                parts="ac")
                for h in range(2):
                    emit_planes(ps[:], w_3, KI // 2, h, xl_3, xh_3,
                                c0, CA, first=False, last=(h == 1),
                                parts="b")
                nc.scalar.copy(cur1[:, m * CA:(m + 1) * CA], ps[:])

            def scan1_block(cur1, c0, ml=0, mh=KH):
                """LIF1 scan; the per-m chains are independent, so callers
                may split the m range to start scanning mid-mm1."""
                cur1_r = cur1[:].rearrange("p (m c) -> p m c", m=KH)
                for t in range(TB):
                    # v computed in-place over the cur1 slice
                    v = cur1_r[:, ml:mh, t * BLOC:(t + 1) * BLOC]
                    ub = u1_3[:, ml:mh,
                              c0 + t * BLOC:c0 + (t + 1) * BLOC]
                    # v = 0.9*y + cur
                    nc.vector.scalar_tensor_tensor(v, y1_3[:, ml:mh, :],
                                                   DECAY, v,
                                                   op.mult, op.add)
                    # u = (v < 1) * 2^-15, e5m2 for the DoubleRow matmul
                    nc.vector.tensor_scalar(ub, v, TH, MOV,
                                            op.is_lt, op.mult)
                    # y = (v<1)*v
                    nc.vector.scalar_tensor_tensor(y1_3[:, ml:mh, :], v,
                                                   TH, v,
                                                   op.is_lt, op.mult)

            # blocks 0-2 interleaved per m-chunk: mm1 rides the progressive
            # wih DMA (3 blocks of PE work per m-chunk outpaces each chunk's
            # DMA, so the PE never starves on the stream); the m 0-7 half of
            # each head scan is emitted mid-stream so the head cur1 buffers
            # free up quickly for the steady blocks
            NHEAD = 3
            cur1_hd = [cur1_pool.tile([128, KH * CA], f32, tag="cur1",
                                      name=f"cur1_hd{b}")
                       for b in range(NHEAD)]
            for m in range(KH):
                for b in range(NHEAD):
                    mm1_block(xhl_3, xhh_3, b * CA, cur1_hd[b], m)
                if m == 8:
                    for b in range(NHEAD):
                        scan1_block(cur1_hd[b], b * CA, 0, 8)
            for b in range(NHEAD):
                scan1_block(cur1_hd[b], b * CA, 8, KH)

            uhi_ready = {}
            for blk in range(NHEAD, NBLK):
                c0 = blk * CA
                xt = x_pool.tile([128, 2 * KI * CA], e5, tag="x")
                xl_3, xh_3 = load_x(xt, c0, CA)
                if blk == NHEAD + 2:
                    # prefetch the first whh half-chunk once the head x
                    # loads are through: it transfers during phase A, ready
                    # long before the phase seam
                    nc.sync.dma_start(wst_pre[:],
                                      whh_d[0:128, 0:(NHH // 2) * 128])
                cur1 = cur1_pool.tile([128, KH * CA], f32, tag="cur1")
                for m in range(KH):
                    mm1_block(xl_3, xh_3, 0, cur1, m)
                scan1_block(cur1, c0)
                # stage the {0,1} copies for superblocks 0 and 1 as soon as
                # their u1 columns exist, so the phase seam has no DVE stall
                if blk == 3:
                    uhi_ready[0] = stage_uhi(*SUPERS[0])
                if blk == 7:
                    uhi_ready[1] = stage_uhi(*SUPERS[1])

        # ---------------- Phase B: mm2 + LIF2 + mm3 + output scan -----------
        with tc.tile_pool(name="wst", bufs=4) as wst_pool, \
             tc.tile_pool(name="cur2", bufs=2) as cur2_pool, \
             tc.tile_pool(name="smallB", bufs=1) as sm_pool, \
             tc.tile_pool(name="cur3", bufs=1) as cur3_pool, \
             tc.tile_pool(name="s3p", bufs=2) as s3_pool, \
             tc.tile_pool(name="ps3", bufs=2, space="PSUM") as ps3_pool, \
             tc.tile_pool(name="pstr", bufs=2, space="PSUM") as pstr_pool:

            who_sb = sm_pool.tile([128, KH * ND6 * 16], e5, tag="who")
            who4 = who_sb[:].rearrange("p (k i m) -> p k i m", k=KH, i=ND6)
            cs_hh = sm_pool.tile([128, KH], f32, tag="cshh")
            cs_ho = sm_pool.tile([OUTPUT_DIM, 1], f32, tag="csho")
            ident_sb = sm_pool.tile([128, 128], f32, tag="ident")

            nc.sync.dma_start(cs_hh[:], cs_hh_d[:, :])
            nc.sync.dma_start(
                who_sb[:].rearrange("p (k f) -> p k f", k=KH),
                who_d[:, :].rearrange("(k p) f -> p k f", p=128))
            nc.sync.dma_start(cs_ho[:], cs_ho_d[:, :])
            nc.sync.dma_start(ident_sb[:], id_d[:, :])

            y2 = sm_pool.tile([128, KH * BLOC], f32, tag="y2")
            yo = sm_pool.tile([OUTPUT_DIM, BLOC], f32, tag="yo")
            vo = sm_pool.tile([OUTPUT_DIM, BLOC], f32, tag="vo")
            acc0 = sm_pool.tile([OUTPUT_DIM, BLOC], f32, tag="acc0")
            acc1 = sm_pool.tile([OUTPUT_DIM, BLOC], f32, tag="acc1")
            acc = [acc0, acc1]
            out_sb = sm_pool.tile([OUTPUT_DIM, BLOC], f32, tag="rate")
            nc.vector.memset(y2[:], 0.0)
            nc.vector.memset(yo[:], 0.0)
            nc.vector.memset(acc[0][:], 0.0)
            y2_3 = y2[:].rearrange("p (m b) -> p m b", m=KH)

            def emit_mm3(c0, uoff, cn):
                """mm3 (transposed: moving side = W_ho planes, 10-wide
                output) + PE transpose back + output-layer scan."""
                cur3 = cur3_pool.tile([OUTPUT_DIM, SCMAX], f32, tag="cur3")
                for ch in range(cn // 128):
                    ps3 = ps3_pool.tile([128, OUTPUT_DIM], f32, tag="ps3")
                    for k in range(KH // 2):
                        # stationary: u2 k-tile pair; moving: W_ho planes
                        ub = u1_3[:, 2 * k:2 * k + 2,
                                  uoff + ch * 128:uoff + (ch + 1) * 128]
                        for i in range(ND6):
                            nc.tensor.matmul(
                                ps3[:],
                                ub,
                                who4[:, 2 * k:2 * k + 2, i,
                                     0:OUTPUT_DIM],
                                start=(k == 0 and i == 0),
                                stop=(k == KH // 2 - 1 and i == ND6 - 1),
                                perf_mode=DR)
                    s3 = s3_pool.tile([128, OUTPUT_DIM], f32, tag="s3")
                    nc.scalar.copy(s3[:], ps3[:])
                    pst = pstr_pool.tile([OUTPUT_DIM, 128], f32, tag="pst")
                    nc.tensor.transpose(pst[:], s3[:], ident_sb[:])
                    # cur3 = colsum_ho - u2@W_ho  (true output current)
                    nc.scalar.activation(cur3[:, ch * 128:(ch + 1) * 128],
                                         pst[:], ident,
                                         bias=cs_ho[:, 0:1], scale=-1.0)
                # output-layer scan runs on the otherwise-idle GPSIMD
                # engine so the tail never serializes behind the DVE scan
                for t in range(cn // BLOC):
                    g = c0 // BLOC + t
                    sl = cur3[:, t * BLOC:(t + 1) * BLOC]
                    nc.vector.scalar_tensor_tensor(vo[:], yo[:], DECAY, sl,
                                                   op.mult, op.add)
                    nc.vector.scalar_tensor_tensor(acc[(g + 1) % 2][:], vo[:],
                                                   TH, acc[g % 2][:],
                                                   op.is_lt, op.add)
                    nc.vector.scalar_tensor_tensor(yo[:], vo[:], TH, vo[:],
                                                   op.is_lt, op.mult)

            prev = None
            for sup, (c0, cn) in enumerate(SUPERS):
                uoff = USLOT[sup] * SCMAX
                uhalves = uhi_ready.pop(sup)
                # chunk the column range so each PSUM tile fits one bank
                # (chunks align with the 320-col uhi staging halves)
                chunks = [(0, 320), (320, 320)] if cn == 640 else [(0, cn)]
                cur2 = cur2_pool.tile([128, KH * SCMAX], f32, tag="cur2")
                cur2_r = cur2[:].rearrange("p (m c) -> p m c", m=KH)
                for m2 in range(KH):
                    if m2 == 8 and prev is not None:
                        emit_mm3(*prev)
                        prev = None
                    if m2 == 8 and sup + 2 < len(SUPERS):
                        # stage the {0,1} copy two superblocks ahead
                        uhi_ready[sup + 2] = stage_uhi(*SUPERS[sup + 2])
                    wst = wst_pool.tile([128, NHH * 128], e4, tag="wst")
                    if sup == 0 and m2 == 0:
                        # k 8-15 only; k 0-7 comes from the prefetched half
                        nc.sync.dma_start(
                            wst[:, (NHH // 2) * 128:],
                            whh_d[0:128, (NHH // 2) * 128:])
                    else:
                        nc.sync.dma_start(
                            wst[:], whh_d[m2 * 128:(m2 + 1) * 128, :])
                    wst_3 = wst[:].rearrange("p (i f) -> p i f", i=NHH)
                    wpre_3 = wst_pre[:].rearrange("p (i f) -> p i f",
                                                  i=NHH // 2)
                    for (off, ncol) in chunks:
                        ps = psB.tile([128, ncol], f32, tag="psB")
                        uh_3 = uhalves[off // UHW]
                        for h in range(2):
                            use_pre = (sup == 0 and m2 == 0 and h == 0)
                            w_3 = wpre_3 if use_pre else wst_3
                            # the prefetch tile holds half 0 at offset 0
                            hh = 0 if use_pre else h
                            emit_planes4(ps[:], w_3, KH // 2, hh, u1_3, uh_3,
                                         c0 + off, ncol,
                                         first=(h == 0), last=(h == 1),
                                         c0h=off % UHW)
                        # cur2 = colsum_hh - u1@W_hh (true layer-2 current)
                        nc.scalar.activation(
                            cur2_r[:, m2, off:off + ncol],
                            ps[:], ident, bias=cs_hh[:, m2:m2 + 1],
                            scale=-1.0)
                for t in range(cn // BLOC):
                    # v computed in-place over the cur2 slice
                    v = cur2_r[:, :, t * BLOC:(t + 1) * BLOC]
                    ub = u1_3[:, :,
                              uoff + t * BLOC:uoff + (t + 1) * BLOC]
                    nc.vector.scalar_tensor_tensor(v, y2_3, DECAY, v,
                                                   op.mult, op.add)
                    nc.vector.tensor_scalar(ub, v, TH, MOV,
                                            op.is_lt, op.mult)
                    nc.vector.scalar_tensor_tensor(y2_3, v, TH, v,
                                                   op.is_lt, op.mult)
                prev = (c0, uoff, cn)
            emit_mm3(*prev)

            # rate = 1 - acc/T   (acc holds sum of u_out; s = 1-u)
            nc.vector.tensor_scalar(out_sb[:], acc[T % 2][:], -1.0 / T, 1.0,
                                    op.mult, op.add)
            nc.sync.dma_start(out_d[:, :], out_sb[:])

    nc.compile()
    return nc


def _digit_planes32(w, nplanes, kbits, pscale):
    """Decompose fp32 weights into nplanes exact e4m3 base-32 digit planes.

    w ~= Wfix * 2^-kbits with Wfix = sum_i d_i 32^i, d_i in [-16,15].
    Plane i holds d_i * pscale[i]; the moving operand supplies the rest of
    each plane's 32^i * 2^-kbits scale so every product is fp32-exact.
    Returns (planes [nplanes, *w.shape] e4m3-exact fp32, effective weights
    fp32)."""
    dmax = 16 * (32 ** nplanes - 1) // 31
    wfix = np.round(w.astype(np.float64) * (1 << kbits)).astype(np.int64)
    assert np.abs(wfix).max() <= dmax, "weights exceed digit range"
    planes = np.zeros((nplanes,) + w.shape, np.float32)
    rem = wfix.copy()
    for i in range(nplanes):
        d = ((rem + 16) % 32) - 16
        rem = (rem - d) >> 5
        planes[i] = d * np.float32(pscale[i])
    assert np.all(rem == 0)
    weff = (wfix * (2.0 ** -kbits)).astype(np.float32)
    return planes, weff


def _digit_planes16(w):
    """Base-16 e5m2 planes for W_ho (moving side of mm3; u2 carries 2^-15).

    w ~= Wfix * 2^-KB23, plane i holds d_i * 2^(4i - KB23 + 15)."""
    wfix = np.round(w.astype(np.float64) * (1 << KB23)).astype(np.int64)
    assert np.abs(wfix).max() <= DMAX16, "weights exceed digit range"
    planes = np.zeros((ND6,) + w.shape, np.float32)
    rem = wfix.copy()
    for i in range(ND6):
        d = ((rem + 8) % 16) - 8
        rem = (rem - d) >> 4
        planes[i] = d * np.float32(2.0 ** (4 * i - KB23 + 15))
    assert np.all(rem == 0)
    weff = (wfix * (2.0 ** -KB23)).astype(np.float32)
    return planes, weff


def _pack_planes(planes, kc, mc, items_fn):
    """Pack [nplanes, K, M] planes into the DMA layout: row (m*128+p) holds
    the nplanes*kc plane-items (two k-halves, DR-pair-adjacent) of 128
    bytes each."""
    nitem = len(planes) * kc
    out = np.zeros((mc, 128, nitem, 128), np.float32)
    for m in range(mc):
        mcols = slice(m * 128, (m + 1) * 128)
        idx = 0
        for h in range(2):
            for (k, pl) in items_fn(kc // 2, h):
                out[m, :, idx, :] = planes[pl][k * 128:(k + 1) * 128, mcols]
                idx += 1
    return out.reshape(mc * 128, nitem * 128)


def kernel(input_bins, W_ih, W_hh, W_ho):
    global _BUILT
    if _BUILT is None:
        _BUILT = _build()
    nc = _BUILT
    import ml_dtypes
    e5np = ml_dtypes.float8_e5m2
    e4np = ml_dtypes.float8_e4m3

    input_bins = np.ascontiguousarray(input_bins, dtype=np.float32)
    W_ih = np.ascontiguousarray(W_ih, dtype=np.float32)
    W_hh2 = np.ascontiguousarray(np.asarray(W_hh)[0], dtype=np.float32)
    W_ho = np.ascontiguousarray(W_ho, dtype=np.float32)

    pih, wih_eff = _digit_planes32(W_ih, NP_IH, KBITS_IH, PSCALE_IH)
    phh, whh_eff = _digit_planes32(W_hh2, NP_HH, KBITS_HH, PSCALE_HH)
    pho, who_eff = _digit_planes16(W_ho)     # [ND6, 2048, 10]

    wihd = np.ascontiguousarray(
        _pack_planes(pih, KI, KH, _half_items5)).astype(e4np)
    whhd = np.ascontiguousarray(
        _pack_planes(phh, KH, KH, _half_items4)).astype(e4np)

    # who planes padded to 16 output cols: [kt*128, dig*16]
    whod = np.zeros((KH, 128, ND6, 16), np.float32)
    whod[:, :, :, :OUTPUT_DIM] = pho.reshape(ND6, KH, 128, OUTPUT_DIM) \
        .transpose(1, 2, 0, 3)
    whod8 = np.ascontiguousarray(whod.reshape(KH * 128, ND6 * 16)).astype(e5np)

    cs_hh = np.ascontiguousarray(
        whh_eff.sum(axis=0, dtype=np.float64).astype(np.float32)
        .reshape(KH, 128).T)
    cs_ho = who_eff.sum(axis=0, dtype=np.float64).astype(np.float32) \
        .reshape(OUTPUT_DIM, 1)

    in_maps = []
    for c in range(NCORES):
        xb = input_bins[c * BLOC:(c + 1) * BLOC]        # [32, 1024, 100]
        xt = xb.transpose(1, 2, 0).reshape(INPUT_DIM, COLS)
        xall = np.empty((2 * INPUT_DIM, COLS), e5np)
        xall[0:INPUT_DIM] = (xt * np.float32(MOV)).astype(e5np)
        xall[INPUT_DIM:] = xt.astype(e5np)
        in_maps.append({
            "xall": np.ascontiguousarray(xall), "wihd": wihd, "whhd": whhd,
            "whod": whod8, "cs_hh": cs_hh, "cs_ho": cs_ho,
            "ident": np.eye(128, dtype=np.float32),
        })

    from concourse.bass_utils import run_bass_kernel_spmd
    res = run_bass_kernel_spmd(nc, in_maps, core_ids=list(range(NCORES)))

    out = np.empty((BATCH, OUTPUT_DIM), dtype=np.float32)
    for c in range(NCORES):
        out[c * BLOC:(c + 1) * BLOC] = res.results[c]["out"].T
    return out


# revision 51
# speedup vs baseline: 1.0021x; 1.0021x over previous
"""Trainium2 Bass kernel for a 2-hidden-layer LIF spiking network.

Math (per timestep t, per layer):
    v = 0.9*y + cur ;  spike s = (v >= 1) ;  y = v*(1-s) = v*u  with u = (v < 1)
Layer currents:
    cur1 = x_t @ W_ih            (x binary, precomputable for ALL t)
    cur2 = s1 @ W_hh = colsum(W_hh) - u1 @ W_hh
    cur3 = s2 @ W_ho = colsum(W_ho) - u2 @ W_ho
Output: rate = mean_t s_out = 1 - sum_t(u_out)/T

Key restructurings:
  * Layer 1's recurrence does not depend on layer 2, so all three matmuls are
    batched over the full (T*B) column space; only the cheap elementwise LIF
    scans are sequential in t.
  * Weights are quantized to fixed point and decomposed into exact signed
    base-32 digit planes stored in fp8 e4m3 (digits in [-16,15] times
    power-of-2 scales are exact in e4m3; 5 bits/plane vs e5m2's 4). Pairs of
    digit planes feed fp8 DoubleRow matmuls (2 stationary planes per
    instruction at 0.5 cycles/row). W_ih gets 5 planes (24-bit, 1.25
    cycles/row): the input layer is the flip-sensitive one. W_hh gets 4
    planes (20-bit, 1.0 cycles/row): measured output flips stay in the same
    singleton class while mm2 -- the dominant PE cost -- drops 1/3 vs the
    e5m2 6-plane scheme. e4m3's narrow exponent range cannot span the full
    digit-scale range against a single moving value, so the low plane pairs
    ride a moving operand carrying 2^-15 while the high pairs ride a second
    moving copy (1.0 for W_ih, 2^-5 for W_hh) with the planes stored
    correspondingly lower. Every product is exact in fp32 PSUM and all
    planes accumulate into ONE PSUM group -- no extra combine ops.
  * The moving tiles are stored once per scale: u1 at 2^-15 stays resident
    in SBUF for ALL timesteps; the hi-scale copy is staged per superblock
    by one cheap DVE op (off the critical scan chain). x ships at both
    scales in one packed DRAM tensor (one DMA per block).

Sharding: data-parallel over batch (256/8 = 32 rows per core), weights
replicated, no cross-core communication.

Per-core schedule:
  Phase A (W_ih digit planes resident, 10.5MB, loaded progressively per
    output chunk; the first THREE blocks are emitted interleaved per
    m-chunk so mm1 rides the weight DMA without starving -- 3 blocks of PE
    work per m-chunk outpaces each chunk's DMA): mm1 over blocks of 5
    steps, LIF1 scan fused per block (v computed in-place over cur1, head
    scans split by m-chunk to free buffers early), u1 written straight
    into its resident SBUF tile (never spilled).
  Phase B (W_hh digit planes streamed from DRAM per 128-col output chunk,
    quad-buffered; the first half-chunk prefetched during phase A into a
    disjoint tile so the phase seam has no DMA stall): superblocks of 20
    steps; mm2 -> cur2 with colsum correction fused into the PSUM->SBUF
    Identity-activation copy (scale=-1, bias=colsum), LIF2 scan (u2
    overwrites u1's own retired ring slot), mm3 transposed (moving side =
    W_ho planes so its cost scales with the 10-wide output, then a PE
    transpose restores [10, cols] via an identity matmul), output LIF scan,
    final rate. mm3 for superblock s is emitted mid-way through superblock
    s+1's mm2 so the PE never waits on the DVE scan.
"""

import numpy as np

# ---- problem constants (hardcoded; kernel.py must be self-contained) ----
BATCH = 256
INPUT_DIM = 1024
HIDDEN_DIM = 2048
OUTPUT_DIM = 10
T = 100
NCORES = 8
BLOC = BATCH // NCORES          # 32 batch rows per core
TB = 5                          # timesteps per phase-A block
NBLK = T // TB                  # 20 blocks
CA = TB * BLOC                  # 160 columns per phase-A block
C = 320                         # columns per phase-B matmul chunk
COLS = T * BLOC                 # 3200 total columns
# phase-B superblocks (col0, ncols); u2 for super s overwrites u1's own
# slot s -- by the time the LIF2 scan emits u2(s), mm2(s) has consumed
# every u1 column in that slot (the scan is already ordered after mm2(s)
# through cur2), so no spare slot is needed.
SCMAX = 640
SUPERS = [(0, 640), (640, 640), (1280, 640), (1920, 640), (2560, 640)]
USLOT = [0, 1, 2, 3, 4]
NSLOT = 5
KI = INPUT_DIM // 128           # 8 k-chunks for mm1
KH = HIDDEN_DIM // 128          # 16 k-chunks (and m-chunks) for mm2
DECAY = 0.9
THRESH = 1.0
TH_NUDGE = 0.0                  # tie-break re-roll knob (harmless ~1e-6 scale)

# base-32 e4m3 digit planes. W_ih gets 5 planes (24-bit fixed point): the
# input layer is the flip-sensitive one (its quantization error enters v1
# directly every step). W_hh gets 4 planes (20-bit): measured flip count
# stays in the same singleton class as 24-bit, and mm2 dominates PE time.
NP_IH = 5
KBITS_IH = 24
# plane i contributes d_i * 32^i * 2^-24; planes 0-2 ride moving 2^-15,
# planes 3,4 ride moving 1.0 (stored 2^15 lower)
PSCALE_IH = [2.0 ** -9, 2.0 ** -4, 2.0 ** 1, 2.0 ** -9, 2.0 ** -4]
NP_HH = 4
KBITS_HH = 20
# plane i contributes d_i * 32^i * 2^-20; planes 0,1 ride moving 2^-15,
# planes 2,3 ride moving 2^-5 (stored 2^10 lower)
PSCALE_HH = [2.0 ** -5, 2.0 ** 0, 2.0 ** -5, 2.0 ** 0]
UHI_VAL = 2.0 ** 10             # u_hi = u_lo * 2^10 = {0, 2^-5}
MOV = 2.0 ** -15                # lo moving value (e5m2 subnormal, exact)
NIH = NP_IH * KI                # 40 plane-items per wih m-chunk
NHH = NP_HH * KH                # 64 plane-items per whh m2-chunk

# mm3 weight planes stay base-16 e5m2 (cost is negligible at 10-wide out)
ND6 = 6
KB23 = 23
DMAX16 = 7 * (16 ** ND6 - 1) // 15

_BUILT = None


def _half_items5(kc2, h):
    """5-plane item order within one half (kc2 k-chunks starting at h*kc2):
    A-pairs (planes 0,1 per k), C-pairs (plane 2 of adjacent k), B-pairs
    (planes 3,4 per k). Returns [(k, plane), ...], DR-pair-adjacent."""
    k0 = h * kc2
    items = []
    for k in range(k0, k0 + kc2):
        items += [(k, 0), (k, 1)]
    for k in range(k0, k0 + kc2):
        items.append((k, 2))
    for k in range(k0, k0 + kc2):
        items += [(k, 3), (k, 4)]
    return items


def _half_items4(kc2, h):
    """4-plane item order: A-pairs (planes 0,1 per k) then B-pairs (planes
    2,3 per k); no cross-k pair needed."""
    k0 = h * kc2
    items = []
    for k in range(k0, k0 + kc2):
        items += [(k, 0), (k, 1)]
    for k in range(k0, k0 + kc2):
        items += [(k, 2), (k, 3)]
    return items


def _build():
    """Trace + compile the Bass program once."""
    from contextlib import ExitStack

    import concourse.bacc as bacc
    import concourse.tile as tile
    from concourse import mybir
    from concourse.alu_op_type import AluOpType as op

    f32 = mybir.dt.float32
    e5 = mybir.dt.float8e5
    e4 = mybir.dt.float8e4
    DR = mybir.MatmulPerfMode.DoubleRow
    ident = mybir.ActivationFunctionType.Identity
    TH = THRESH + TH_NUDGE

    nc = bacc.Bacc("TRN2", target_bir_lowering=False, debug=False,
                   num_devices=NCORES)

    # x at both moving scales packed in one tensor (lo rows then hi rows):
    # [2*input_dim, t*b] t-major columns -- one DMA covers both scales
    x_d = nc.dram_tensor("xall", [2 * INPUT_DIM, COLS], e5,
                         kind="ExternalInput").ap()
    xlo_d = x_d[0:INPUT_DIM, :]
    xhi_d = x_d[INPUT_DIM:2 * INPUT_DIM, :]
    # wih planes: row (m*128+p) holds that partition's NIH plane-items
    wih_d = nc.dram_tensor("wihd", [KH * 128, NIH * 128], e4,
                           kind="ExternalInput").ap()
    # whh planes: row (m2*128+p) holds NHH plane-items (two k-halves)
    whh_d = nc.dram_tensor("whhd", [KH * 128, NHH * 128], e4,
                           kind="ExternalInput").ap()
    # who planes padded to 16 cols: [kt*128, dig*16] (e5m2 base-16)
    who_d = nc.dram_tensor("whod", [KH * 128, ND6 * 16], e5,
                           kind="ExternalInput").ap()
    cs_hh_d = nc.dram_tensor("cs_hh", [128, KH], f32, kind="ExternalInput").ap()
    cs_ho_d = nc.dram_tensor("cs_ho", [OUTPUT_DIM, 1], f32,
                             kind="ExternalInput").ap()
    id_d = nc.dram_tensor("ident", [128, 128], f32, kind="ExternalInput").ap()
    out_d = nc.dram_tensor("out", [OUTPUT_DIM, BLOC], f32,
                           kind="ExternalOutput").ap()

    with tile.TileContext(nc) as tc, ExitStack() as ctx:
        # spike complements {0, 2^-15}, resident across both phases
        # [p, kt, col]: cols 0..3200 hold u1; 6 ring slots of 640 also serve
        # as u2 staging (a slot is reused once mm2 has consumed its u1 cols)
        u1_pool = ctx.enter_context(tc.tile_pool(name="u1", bufs=1))
        u1 = u1_pool.tile([128, KH * NSLOT * SCMAX], e5, tag="u1")
        u1_3 = u1[:].rearrange("p (k c) -> p k c", k=KH)

        # {0,1} copies of u1, staged per superblock for the hi-plane pairs
        # (two 320-col halves per superblock, two superblocks in flight)
        uhi_pool = ctx.enter_context(tc.tile_pool(name="uhi", bufs=4))

        UHW = SCMAX // 2

        def stage_uhi(c0, cn):
            """Stage {0,1} copies of u1 cols [c0, c0+cn) as 320-col half
            tiles (halves the pool footprint vs full-superblock tiles)."""
            halves = []
            for hc0 in range(0, cn, UHW):
                w = min(UHW, cn - hc0)
                uh = uhi_pool.tile([128, KH * UHW], e5, tag="uhi",
                                   name=f"uhi_{c0 + hc0}")
                uh_3 = uh[:].rearrange("p (k c) -> p k c", k=KH)
                nc.vector.tensor_scalar(uh_3[:, :, 0:w],
                                        u1_3[:, :, c0 + hc0:c0 + hc0 + w],
                                        UHI_VAL, None, op.mult)
                halves.append(uh_3)
            return halves

        # first whh half-chunk (k 0-7 of m2=0), prefetched into space
        # disjoint from phase A's pools so its DMA runs during phase A
        wpre_pool = ctx.enter_context(tc.tile_pool(name="wpre", bufs=1))
        wst_pre = wpre_pool.tile([128, (NHH // 2) * 128], e4, tag="wpre")

        def emit_planes(ps, w_3, kc2, h, mov_lo, mov_hi, c0, cn, first, last,
                        c0h=None, parts="all"):
            """One half's 2.5*kc2 DoubleRow matmuls into PSUM group `ps`.
            w_3: stationary tile viewed [p, item, 128] (items for half h at
            item offset h*5*kc2). mov_lo/mov_hi: [p, k, c] moving tiles;
            c0h: column origin within mov_hi (defaults to c0). parts
            selects the lo-moving pairs ("ac"), hi-moving pairs ("b"), or
            everything ("all") so callers can front-load the work that only
            needs mov_lo while mov_hi's DMA is still in flight."""
            if c0h is None:
                c0h = c0
            base = h * 5 * kc2
            k0 = h * kc2
            idx = 0
            if parts in ("all", "ac"):
                for k in range(k0, k0 + kc2):    # A-pairs (planes 0,1)
                    nc.tensor.matmul(
                        ps, w_3[:, base + idx:base + idx + 2, :],
                        mov_lo[:, k, c0:c0 + cn].unsqueeze(1)
                        .broadcast_to([128, 2, cn]),
                        start=(first and idx == 0), stop=False, perf_mode=DR)
                    idx += 2
                for kk in range(kc2 // 2):       # C-pairs (plane 2, k/k+1)
                    nc.tensor.matmul(
                        ps, w_3[:, base + idx:base + idx + 2, :],
                        mov_lo[:, k0 + 2 * kk:k0 + 2 * kk + 2, c0:c0 + cn],
                        start=False, stop=False, perf_mode=DR)
                    idx += 2
            else:
                idx = 3 * kc2
            if parts in ("all", "b"):
                for k in range(k0, k0 + kc2):    # B-pairs (planes 3,4)
                    nc.tensor.matmul(
                        ps, w_3[:, base + idx:base + idx + 2, :],
                        mov_hi[:, k, c0h:c0h + cn].unsqueeze(1)
                        .broadcast_to([128, 2, cn]),
                        start=False, stop=(last and idx == 5 * kc2 - 2),
                        perf_mode=DR)
                    idx += 2

        def emit_planes4(ps, w_3, kc2, h, mov_lo, mov_hi, c0, cn, first,
                         last, c0h):
            """4-plane variant: 2*kc2 DoubleRow matmuls (A-pairs vs mov_lo,
            B-pairs vs mov_hi), items at offset h*4*kc2."""
            base = h * 4 * kc2
            k0 = h * kc2
            idx = 0
            for k in range(k0, k0 + kc2):        # A-pairs (planes 0,1)
                nc.tensor.matmul(
                    ps, w_3[:, base + idx:base + idx + 2, :],
                    mov_lo[:, k, c0:c0 + cn].unsqueeze(1)
                    .broadcast_to([128, 2, cn]),
                    start=(first and idx == 0), stop=False, perf_mode=DR)
                idx += 2
            for k in range(k0, k0 + kc2):        # B-pairs (planes 2,3)
                nc.tensor.matmul(
                    ps, w_3[:, base + idx:base + idx + 2, :],
                    mov_hi[:, k, c0h:c0h + cn].unsqueeze(1)
                    .broadcast_to([128, 2, cn]),
                    start=False, stop=(last and idx == 4 * kc2 - 2),
                    perf_mode=DR)
                idx += 2

        # ---------------- Phase A: mm1 + LIF1 scan ----------------
        with tc.tile_pool(name="wih", bufs=1) as wih_pool, \
             tc.tile_pool(name="xin", bufs=2) as x_pool, \
             tc.tile_pool(name="cur1", bufs=3) as cur1_pool, \
             tc.tile_pool(name="st1", bufs=1) as st1_pool, \
             tc.tile_pool(name="psA", bufs=6, space="PSUM") as psA:

            def load_x(xt, c0, ncols):
                # one DMA covers both scales (lo then hi in the free dim)
                xt_4 = xt[:].rearrange("p (s k c) -> p s k c", s=2, k=KI)
                nc.sync.dma_start(
                    xt_4,
                    x_d[:, c0:c0 + ncols].rearrange("(s k p) c -> p s k c",
                                                    p=128, s=2))
                return xt_4[:, 0], xt_4[:, 1]

            # wih digit planes, per m-chunk (progressive: mm1 m-chunk can
            # start as soon as its planes land); the m=0 chunk issues first,
            # then the two head x blocks, then the rest of the chunks
            def load_wih(m):
                w = wih_pool.tile([128, NIH * 128], e4, tag=f"wih_{m}")
                nc.sync.dma_start(w[:], wih_d[m * 128:(m + 1) * 128, :])
                return w

            # first wih half-load, then head-x lo, then the second half,
            # then head-x hi: the first A-pair matmuls start after just two
            # small DMAs instead of the whole m=0 chunk + both x tensors
            w0 = wih_pool.tile([128, NIH * 128], e4, tag="wih_0")
            halfb = (NIH // 2) * 128
            nc.sync.dma_start(w0[:, 0:halfb], wih_d[0:128, 0:halfb])
            x_head = x_pool.tile([128, 2 * KI * 3 * CA], e5, tag="xh",
                                 bufs=1)
            xh_4 = x_head[:].rearrange("p (s k c) -> p s k c", s=2, k=KI)
            nc.sync.dma_start(
                xh_4[:, 0],
                xlo_d[:, 0:3 * CA].rearrange("(k p) c -> p k c", p=128))
            nc.sync.dma_start(w0[:, halfb:], wih_d[0:128, halfb:])
            nc.sync.dma_start(
                xh_4[:, 1],
                xhi_d[:, 0:3 * CA].rearrange("(k p) c -> p k c", p=128))
            xhl_3, xhh_3 = xh_4[:, 0], xh_4[:, 1]
            wih_sb = [w0]
            for m in range(1, KH):
                wih_sb.append(load_wih(m))

            y1 = st1_pool.tile([128, KH * BLOC], f32, tag="y1")
            nc.vector.memset(y1[:], 0.0)
            y1_3 = y1[:].rearrange("p (m b) -> p m b", m=KH)

            def mm1_block(xl_3, xh_3, c0, cur1, m):
                ps = psA.tile([128, CA], f32, tag="psA")
                w_3 = wih_sb[m][:].rearrange("p (i f) -> p i f", i=NIH)
                # lo-moving pairs of both halves first: the x_hi DMA (the
                # later half of each x load) is never on the critical path
                for h in range(2):
                    emit_planes(ps[:], w_3, KI // 2, h, xl_3, xh_3,
                                c0, CA, first=(h == 0), last=False,
                                parts="ac")
                for h in range(2):
                    emit_planes(ps[:], w_3, KI // 2, h, xl_3, xh_3,
                                c0, CA, first=False, last=(h == 1),
                                parts="b")
                nc.scalar.copy(cur1[:, m * CA:(m + 1) * CA], ps[:])

            def scan1_block(cur1, c0, ml=0, mh=KH):
                """LIF1 scan; the per-m chains are independent, so callers
                may split the m range to start scanning mid-mm1."""
                cur1_r = cur1[:].rearrange("p (m c) -> p m c", m=KH)
                for t in range(TB):
                    # v computed in-place over the cur1 slice
                    v = cur1_r[:, ml:mh, t * BLOC:(t + 1) * BLOC]
                    ub = u1_3[:, ml:mh,
                              c0 + t * BLOC:c0 + (t + 1) * BLOC]
                    # v = 0.9*y + cur
                    nc.vector.scalar_tensor_tensor(v, y1_3[:, ml:mh, :],
                                                   DECAY, v,
                                                   op.mult, op.add)
                    # u = (v < 1) * 2^-15, e5m2 for the DoubleRow matmul
                    nc.vector.tensor_scalar(ub, v, TH, MOV,
                                            op.is_lt, op.mult)
                    # y = (v<1)*v
                    nc.vector.scalar_tensor_tensor(y1_3[:, ml:mh, :], v,
                                                   TH, v,
                                                   op.is_lt, op.mult)

            # blocks 0-2 interleaved per m-chunk: mm1 rides the progressive
            # wih DMA (3 blocks of PE work per m-chunk outpaces each chunk's
            # DMA, so the PE never starves on the stream); the m 0-7 half of
            # each head scan is emitted mid-stream so the head cur1 buffers
            # free up quickly for the steady blocks
            NHEAD = 3
            cur1_hd = [cur1_pool.tile([128, KH * CA], f32, tag="cur1",
                                      name=f"cur1_hd{b}")
                       for b in range(NHEAD)]
            for m in range(KH):
                for b in range(NHEAD):
                    mm1_block(xhl_3, xhh_3, b * CA, cur1_hd[b], m)
                if m == 8:
                    for b in range(NHEAD):
                        scan1_block(cur1_hd[b], b * CA, 0, 8)
            for b in range(NHEAD):
                scan1_block(cur1_hd[b], b * CA, 8, KH)

            uhi_ready = {}
            for blk in range(NHEAD, NBLK):
                c0 = blk * CA
                xt = x_pool.tile([128, 2 * KI * CA], e5, tag="x")
                xl_3, xh_3 = load_x(xt, c0, CA)
                if blk == NHEAD + 2:
                    # prefetch the first whh half-chunk once the head x
                    # loads are through: it transfers during phase A, ready
                    # long before the phase seam
                    nc.sync.dma_start(wst_pre[:],
                                      whh_d[0:128, 0:(NHH // 2) * 128])
                cur1 = cur1_pool.tile([128, KH * CA], f32, tag="cur1")
                for m in range(KH):
                    mm1_block(xl_3, xh_3, 0, cur1, m)
                scan1_block(cur1, c0)
                # stage the {0,1} copies for superblocks 0 and 1 as soon as
                # their u1 columns exist, so the phase seam has no DVE stall
                if blk == 3:
                    uhi_ready[0] = stage_uhi(*SUPERS[0])
                if blk == 7:
                    uhi_ready[1] = stage_uhi(*SUPERS[1])

        # ---------------- Phase B: mm2 + LIF2 + mm3 + output scan -----------
        with tc.tile_pool(name="wst", bufs=4) as wst_pool, \
             tc.tile_pool(name="cur2", bufs=2) as cur2_pool, \
             tc.tile_pool(name="smallB", bufs=1) as sm_pool, \
             tc.tile_pool(name="cur3", bufs=1) as cur3_pool, \
             tc.tile_pool(name="s3p", bufs=2) as s3_pool, \
             tc.tile_pool(name="psB", bufs=4, space="PSUM") as psB, \
             tc.tile_pool(name="ps3", bufs=2, space="PSUM") as ps3_pool, \
             tc.tile_pool(name="pstr", bufs=2, space="PSUM") as pstr_pool:

            who_sb = sm_pool.tile([128, KH * ND6 * 16], e5, tag="who")
            who4 = who_sb[:].rearrange("p (k i m) -> p k i m", k=KH, i=ND6)
            cs_hh = sm_pool.tile([128, KH], f32, tag="cshh")
            cs_ho = sm_pool.tile([OUTPUT_DIM, 1], f32, tag="csho")
            ident_sb = sm_pool.tile([128, 128], f32, tag="ident")

            nc.sync.dma_start(cs_hh[:], cs_hh_d[:, :])
            nc.sync.dma_start(
                who_sb[:].rearrange("p (k f) -> p k f", k=KH),
                who_d[:, :].rearrange("(k p) f -> p k f", p=128))
            nc.sync.dma_start(cs_ho[:], cs_ho_d[:, :])
            nc.sync.dma_start(ident_sb[:], id_d[:, :])

            y2 = sm_pool.tile([128, KH * BLOC], f32, tag="y2")
            yo = sm_pool.tile([OUTPUT_DIM, BLOC], f32, tag="yo")
            vo = sm_pool.tile([OUTPUT_DIM, BLOC], f32, tag="vo")
            acc0 = sm_pool.tile([OUTPUT_DIM, BLOC], f32, tag="acc0")
            acc1 = sm_pool.tile([OUTPUT_DIM, BLOC], f32, tag="acc1")
            acc = [acc0, acc1]
            out_sb = sm_pool.tile([OUTPUT_DIM, BLOC], f32, tag="rate")
            nc.vector.memset(y2[:], 0.0)
            nc.vector.memset(yo[:], 0.0)
            nc.vector.memset(acc[0][:], 0.0)
            y2_3 = y2[:].rearrange("p (m b) -> p m b", m=KH)

            def emit_mm3(c0, uoff, cn):
                """mm3 (transposed: moving side = W_ho planes, 10-wide
                output) + PE transpose back + output-layer scan."""
                cur3 = cur3_pool.tile([OUTPUT_DIM, SCMAX], f32, tag="cur3")
                for ch in range(cn // 128):
                    ps3 = ps3_pool.tile([128, OUTPUT_DIM], f32, tag="ps3")
                    for k in range(KH // 2):
                        # stationary: u2 k-tile pair; moving: W_ho planes
                        ub = u1_3[:, 2 * k:2 * k + 2,
                                  uoff + ch * 128:uoff + (ch + 1) * 128]
                        for i in range(ND6):
                            nc.tensor.matmul(
                                ps3[:],
                                ub,
                                who4[:, 2 * k:2 * k + 2, i,
                                     0:OUTPUT_DIM],
                                start=(k == 0 and i == 0),
                                stop=(k == KH // 2 - 1 and i == ND6 - 1),
                                perf_mode=DR)
                    s3 = s3_pool.tile([128, OUTPUT_DIM], f32, tag="s3")
                    nc.scalar.copy(s3[:], ps3[:])
                    pst = pstr_pool.tile([OUTPUT_DIM, 128], f32, tag="pst")
                    nc.tensor.transpose(pst[:], s3[:], ident_sb[:])
                    # cur3 = colsum_ho - u2@W_ho  (true output current)
                    nc.scalar.activation(cur3[:, ch * 128:(ch + 1) * 128],
                                         pst[:], ident,
                                         bias=cs_ho[:, 0:1], scale=-1.0)
                # output-layer scan runs on the otherwise-idle GPSIMD
                # engine so the tail never serializes behind the DVE scan
                for t in range(cn // BLOC):
                    g = c0 // BLOC + t
                    sl = cur3[:, t * BLOC:(t + 1) * BLOC]
                    nc.vector.scalar_tensor_tensor(vo[:], yo[:], DECAY, sl,
                                                   op.mult, op.add)
                    nc.vector.scalar_tensor_tensor(acc[(g + 1) % 2][:], vo[:],
                                                   TH, acc[g % 2][:],
                                                   op.is_lt, op.add)
                    nc.vector.scalar_tensor_tensor(yo[:], vo[:], TH, vo[:],
                                                   op.is_lt, op.mult)

            prev = None
            for sup, (c0, cn) in enumerate(SUPERS):
                uoff = USLOT[sup] * SCMAX
                uhalves = uhi_ready.pop(sup)
                # chunk the column range so each PSUM tile fits one bank
                # (chunks align with the 320-col uhi staging halves)
                chunks = [(0, 320), (320, 320)] if cn == 640 else [(0, cn)]
                cur2 = cur2_pool.tile([128, KH * SCMAX], f32, tag="cur2")
                cur2_r = cur2[:].rearrange("p (m c) -> p m c", m=KH)
                for m2 in range(KH):
                    if m2 == 8 and prev is not None:
                        emit_mm3(*prev)
                        prev = None
                    if m2 == 8 and sup + 2 < len(SUPERS):
                        # stage the {0,1} copy two superblocks ahead
                        uhi_ready[sup + 2] = stage_uhi(*SUPERS[sup + 2])
                    wst = wst_pool.tile([128, NHH * 128], e4, tag="wst")
                    if sup == 0 and m2 == 0:
                        # k 8-15 only; k 0-7 comes from the prefetched half
                        nc.sync.dma_start(
                            wst[:, (NHH // 2) * 128:],
                            whh_d[0:128, (NHH // 2) * 128:])
                    else:
                        nc.sync.dma_start(
                            wst[:], whh_d[m2 * 128:(m2 + 1) * 128, :])
                    wst_3 = wst[:].rearrange("p (i f) -> p i f", i=NHH)
                    wpre_3 = wst_pre[:].rearrange("p (i f) -> p i f",
                                                  i=NHH // 2)
                    for (off, ncol) in chunks:
                        ps = psB.tile([128, ncol], f32, tag="psB")
                        uh_3 = uhalves[off // UHW]
                        for h in range(2):
                            use_pre = (sup == 0 and m2 == 0 and h == 0)
                            w_3 = wpre_3 if use_pre else wst_3
                            # the prefetch tile holds half 0 at offset 0
                            hh = 0 if use_pre else h
                            emit_planes4(ps[:], w_3, KH // 2, hh, u1_3, uh_3,
                                         c0 + off, ncol,
                                         first=(h == 0), last=(h == 1),
                                         c0h=off % UHW)
                        # cur2 = colsum_hh - u1@W_hh (true layer-2 current)
                        nc.scalar.activation(
                            cur2_r[:, m2, off:off + ncol],
                            ps[:], ident, bias=cs_hh[:, m2:m2 + 1],
                            scale=-1.0)
                for t in range(cn // BLOC):
                    # v computed in-place over the cur2 slice
                    v = cur2_r[:, :, t * BLOC:(t + 1) * BLOC]
                    ub = u1_3[:, :,
                              uoff + t * BLOC:uoff + (t + 1) * BLOC]
                    nc.vector.scalar_tensor_tensor(v, y2_3, DECAY, v,
                                                   op.mult, op.add)
                    nc.vector.tensor_scalar(ub, v, TH, MOV,
                                            op.is_lt, op.mult)
                    nc.vector.scalar_tensor_tensor(y2_3, v, TH, v,
                                                   op.is_lt, op.mult)
                prev = (c0, uoff, cn)
            emit_mm3(*prev)

            # rate = 1 - acc/T   (acc holds sum of u_out; s = 1-u)
            nc.vector.tensor_scalar(out_sb[:], acc[T % 2][:], -1.0 / T, 1.0,
                                    op.mult, op.add)
            nc.sync.dma_start(out_d[:, :], out_sb[:])

    nc.compile()
    return nc


def _digit_planes32(w, nplanes, kbits, pscale):
    """Decompose fp32 weights into nplanes exact e4m3 base-32 digit planes.

    w ~= Wfix * 2^-kbits with Wfix = sum_i d_i 32^i, d_i in [-16,15].
    Plane i holds d_i * pscale[i]; the moving operand supplies the rest of
    each plane's 32^i * 2^-kbits scale so every product is fp32-exact.
    Returns (planes [nplanes, *w.shape] e4m3-exact fp32, effective weights
    fp32)."""
    dmax = 16 * (32 ** nplanes - 1) // 31
    wfix = np.round(w.astype(np.float64) * (1 << kbits)).astype(np.int64)
    assert np.abs(wfix).max() <= dmax, "weights exceed digit range"
    planes = np.zeros((nplanes,) + w.shape, np.float32)
    rem = wfix.copy()
    for i in range(nplanes):
        d = ((rem + 16) % 32) - 16
        rem = (rem - d) >> 5
        planes[i] = d * np.float32(pscale[i])
    assert np.all(rem == 0)
    weff = (wfix * (2.0 ** -kbits)).astype(np.float32)
    return planes, weff


def _digit_planes16(w):
    """Base-16 e5m2 planes for W_ho (moving side of mm3; u2 carries 2^-15).

    w ~= Wfix * 2^-KB23, plane i holds d_i * 2^(4i - KB23 + 15)."""
    wfix = np.round(w.astype(np.float64) * (1 << KB23)).astype(np.int64)
    assert np.abs(wfix).max() <= DMAX16, "weights exceed digit range"
    planes = np.zeros((ND6,) + w.shape, np.float32)
    rem = wfix.copy()
    for i in range(ND6):
        d = ((rem + 8) % 16) - 8
        rem = (rem - d) >> 4
        planes[i] = d * np.float32(2.0 ** (4 * i - KB23 + 15))
    assert np.all(rem == 0)
    weff = (wfix * (2.0 ** -KB23)).astype(np.float32)
    return planes, weff


def _pack_planes(planes, kc, mc, items_fn):
    """Pack [nplanes, K, M] planes into the DMA layout: row (m*128+p) holds
    the nplanes*kc plane-items (two k-halves, DR-pair-adjacent) of 128
    bytes each."""
    nitem = len(planes) * kc
    out = np.zeros((mc, 128, nitem, 128), np.float32)
    for m in range(mc):
        mcols = slice(m * 128, (m + 1) * 128)
        idx = 0
        for h in range(2):
            for (k, pl) in items_fn(kc // 2, h):
                out[m, :, idx, :] = planes[pl][k * 128:(k + 1) * 128, mcols]
                idx += 1
    return out.reshape(mc * 128, nitem * 128)


def kernel(input_bins, W_ih, W_hh, W_ho):
    global _BUILT
    if _BUILT is None:
        _BUILT = _build()
    nc = _BUILT
    import ml_dtypes
    e5np = ml_dtypes.float8_e5m2
    e4np = ml_dtypes.float8_e4m3

    input_bins = np.ascontiguousarray(input_bins, dtype=np.float32)
    W_ih = np.ascontiguousarray(W_ih, dtype=np.float32)
    W_hh2 = np.ascontiguousarray(np.asarray(W_hh)[0], dtype=np.float32)
    W_ho = np.ascontiguousarray(W_ho, dtype=np.float32)

    pih, wih_eff = _digit_planes32(W_ih, NP_IH, KBITS_IH, PSCALE_IH)
    phh, whh_eff = _digit_planes32(W_hh2, NP_HH, KBITS_HH, PSCALE_HH)
    pho, who_eff = _digit_planes16(W_ho)     # [ND6, 2048, 10]

    wihd = np.ascontiguousarray(
        _pack_planes(pih, KI, KH, _half_items5)).astype(e4np)
    whhd = np.ascontiguousarray(
        _pack_planes(phh, KH, KH, _half_items4)).astype(e4np)

    # who planes padded to 16 output cols: [kt*128, dig*16]
    whod = np.zeros((KH, 128, ND6, 16), np.float32)
    whod[:, :, :, :OUTPUT_DIM] = pho.reshape(ND6, KH, 128, OUTPUT_DIM) \
        .transpose(1, 2, 0, 3)
    whod8 = np.ascontiguousarray(whod.reshape(KH * 128, ND6 * 16)).astype(e5np)

    cs_hh = np.ascontiguousarray(
        whh_eff.sum(axis=0, dtype=np.float64).astype(np.float32)
        .reshape(KH, 128).T)
    cs_ho = who_eff.sum(axis=0, dtype=np.float64).astype(np.float32) \
        .reshape(OUTPUT_DIM, 1)

    in_maps = []
    for c in range(NCORES):
        xb = input_bins[c * BLOC:(c + 1) * BLOC]        # [32, 1024, 100]
        xt = xb.transpose(1, 2, 0).reshape(INPUT_DIM, COLS)
        xall = np.empty((2 * INPUT_DIM, COLS), e5np)
        xall[0:INPUT_DIM] = (xt * np.float32(MOV)).astype(e5np)
        xall[INPUT_DIM:] = xt.astype(e5np)
        in_maps.append({
            "xall": np.ascontiguousarray(xall), "wihd": wihd, "whhd": whhd,
            "whod": whod8, "cs_hh": cs_hh, "cs_ho": cs_ho,
            "ident": np.eye(128, dtype=np.float32),
        })

    from concourse.bass_utils import run_bass_kernel_spmd
    res = run_bass_kernel_spmd(nc, in_maps, core_ids=list(range(NCORES)))

    out = np.empty((BATCH, OUTPUT_DIM), dtype=np.float32)
    for c in range(NCORES):
        out[c * BLOC:(c + 1) * BLOC] = res.results[c]["out"].T
    return out


# revision 56
# speedup vs baseline: 1.0112x; 1.0091x over previous
"""Trainium2 Bass kernel for a 2-hidden-layer LIF spiking network.

Math (per timestep t, per layer):
    v = 0.9*y + cur ;  spike s = (v >= 1) ;  y = v*(1-s) = v*u  with u = (v < 1)
Layer currents:
    cur1 = x_t @ W_ih            (x binary, precomputable for ALL t)
    cur2 = s1 @ W_hh = colsum(W_hh) - u1 @ W_hh
    cur3 = s2 @ W_ho = colsum(W_ho) - u2 @ W_ho
Output: rate = mean_t s_out = 1 - sum_t(u_out)/T

Key restructurings:
  * Layer 1's recurrence does not depend on layer 2, so all three matmuls are
    batched over the full (T*B) column space; only the cheap elementwise LIF
    scans are sequential in t.
  * Weights are quantized to fixed point and decomposed into exact signed
    base-32 digit planes stored in fp8 e4m3 (digits in [-16,15] times
    power-of-2 scales are exact in e4m3; 5 bits/plane vs e5m2's 4). Pairs of
    digit planes feed fp8 DoubleRow matmuls (2 stationary planes per
    instruction at 0.5 cycles/row). W_ih gets 5 planes (24-bit, 1.25
    cycles/row): the input layer is the flip-sensitive one. W_hh gets 4
    planes (20-bit, 1.0 cycles/row): measured output flips stay in the same
    singleton class while mm2 -- the dominant PE cost -- drops 1/3 vs the
    e5m2 6-plane scheme. e4m3's narrow exponent range cannot span the full
    digit-scale range against a single moving value, so the low plane pairs
    ride a moving operand carrying 2^-15 while the high pairs ride a second
    moving copy (1.0 for W_ih, 2^-5 for W_hh) with the planes stored
    correspondingly lower. Every product is exact in fp32 PSUM and all
    planes accumulate into ONE PSUM group -- no extra combine ops.
  * The moving tiles are stored once per scale: u1 at 2^-15 stays resident
    in SBUF for ALL timesteps; the hi-scale copy is staged per superblock
    by one cheap DVE op (off the critical scan chain). x ships at both
    scales in one packed DRAM tensor (one DMA per block).

Sharding: data-parallel over batch (256/8 = 32 rows per core), weights
replicated, no cross-core communication.

Per-core schedule:
  Phase A (W_ih digit planes resident, 10.5MB, loaded progressively per
    output chunk; the first THREE blocks are emitted interleaved per
    m-chunk so mm1 rides the weight DMA without starving -- 3 blocks of PE
    work per m-chunk outpaces each chunk's DMA): mm1 over blocks of 5
    steps, LIF1 scan fused per block (v computed in-place over cur1, head
    scans split by m-chunk to free buffers early), u1 written straight
    into its resident SBUF tile (never spilled).
  Phase B (W_hh digit planes streamed from DRAM per 128-col output chunk,
    quad-buffered; the first half-chunk prefetched during phase A into a
    disjoint tile so the phase seam has no DMA stall): superblocks of 20
    steps; mm2 -> cur2 with colsum correction fused into the PSUM->SBUF
    Identity-activation copy (scale=-1, bias=colsum), LIF2 scan (u2
    overwrites u1's own retired ring slot), mm3 transposed (moving side =
    W_ho planes so its cost scales with the 10-wide output, then a PE
    transpose restores [10, cols] via an identity matmul), output LIF scan,
    final rate. mm3 for superblock s is emitted mid-way through superblock
    s+1's mm2 so the PE never waits on the DVE scan.
"""

import numpy as np

# ---- problem constants (hardcoded; kernel.py must be self-contained) ----
BATCH = 256
INPUT_DIM = 1024
HIDDEN_DIM = 2048
OUTPUT_DIM = 10
T = 100
NCORES = 8
BLOC = BATCH // NCORES          # 32 batch rows per core
TB = 5                          # timesteps per phase-A block
NBLK = T // TB                  # 20 blocks
CA = TB * BLOC                  # 160 columns per phase-A block
C = 320                         # columns per phase-B matmul chunk
COLS = T * BLOC                 # 3200 total columns
# phase-B superblocks (col0, ncols); u2 for super s overwrites u1's own
# slot s -- by the time the LIF2 scan emits u2(s), mm2(s) has consumed
# every u1 column in that slot (the scan is already ordered after mm2(s)
# through cur2), so no spare slot is needed.
SCMAX = 640
SUPERS = [(0, 640), (640, 640), (1280, 640), (1920, 768), (2688, 512)]
NSLOT = 5
KI = INPUT_DIM // 128           # 8 k-chunks for mm1
KH = HIDDEN_DIM // 128          # 16 k-chunks (and m-chunks) for mm2
DECAY = 0.9
THRESH = 1.0
TH_NUDGE = 0.0                  # tie-break re-roll knob (harmless ~1e-6 scale)

# base-32 e4m3 digit planes. W_ih gets 5 planes (24-bit fixed point): the
# input layer is the flip-sensitive one (its quantization error enters v1
# directly every step). W_hh gets 4 planes (20-bit): measured flip count
# stays in the same singleton class as 24-bit, and mm2 dominates PE time.
NP_IH = 5
KBITS_IH = 24
# plane i contributes d_i * 32^i * 2^-24; planes 0-2 ride moving 2^-15,
# planes 3,4 ride moving 1.0 (stored 2^15 lower)
PSCALE_IH = [2.0 ** -9, 2.0 ** -4, 2.0 ** 1, 2.0 ** -9, 2.0 ** -4]
NP_HH = 4
KBITS_HH = 20
# plane i contributes d_i * 32^i * 2^-20; planes 0,1 ride moving 2^-15,
# planes 2,3 ride moving 2^-5 (stored 2^10 lower)
PSCALE_HH = [2.0 ** -5, 2.0 ** 0, 2.0 ** -5, 2.0 ** 0]
UHI_VAL = 2.0 ** 10             # u_hi = u_lo * 2^10 = {0, 2^-5}
MOV = 2.0 ** -15                # lo moving value (e5m2 subnormal, exact)
NIH = NP_IH * KI                # 40 plane-items per wih m-chunk
NHH = NP_HH * KH                # 64 plane-items per whh m2-chunk

# mm3 weight planes stay base-16 e5m2 (cost is negligible at 10-wide out)
ND6 = 6
KB23 = 23
DMAX16 = 7 * (16 ** ND6 - 1) // 15

_BUILT = None


def _half_items5(kc2, h):
    """5-plane item order within one half (kc2 k-chunks starting at h*kc2):
    A-pairs (planes 0,1 per k), C-pairs (plane 2 of adjacent k), B-pairs
    (planes 3,4 per k). Returns [(k, plane), ...], DR-pair-adjacent."""
    k0 = h * kc2
    items = []
    for k in range(k0, k0 + kc2):
        items += [(k, 0), (k, 1)]
    for k in range(k0, k0 + kc2):
        items.append((k, 2))
    for k in range(k0, k0 + kc2):
        items += [(k, 3), (k, 4)]
    return items


def _half_items4(kc2, h):
    """4-plane item order: A-pairs (planes 0,1 per k) then B-pairs (planes
    2,3 per k); no cross-k pair needed."""
    k0 = h * kc2
    items = []
    for k in range(k0, k0 + kc2):
        items += [(k, 0), (k, 1)]
    for k in range(k0, k0 + kc2):
        items += [(k, 2), (k, 3)]
    return items


def _build():
    """Trace + compile the Bass program once."""
    from contextlib import ExitStack

    import concourse.bacc as bacc
    import concourse.tile as tile
    from concourse import mybir
    from concourse.alu_op_type import AluOpType as op

    f32 = mybir.dt.float32
    e5 = mybir.dt.float8e5
    e4 = mybir.dt.float8e4
    DR = mybir.MatmulPerfMode.DoubleRow
    ident = mybir.ActivationFunctionType.Identity
    TH = THRESH + TH_NUDGE

    nc = bacc.Bacc("TRN2", target_bir_lowering=False, debug=False,
                   num_devices=NCORES)

    # x at both moving scales packed in one tensor (lo rows then hi rows):
    # [2*input_dim, t*b] t-major columns -- one DMA covers both scales
    x_d = nc.dram_tensor("xall", [2 * INPUT_DIM, COLS], e5,
                         kind="ExternalInput").ap()
    xlo_d = x_d[0:INPUT_DIM, :]
    xhi_d = x_d[INPUT_DIM:2 * INPUT_DIM, :]
    # wih planes: row (m*128+p) holds that partition's NIH plane-items
    wih_d = nc.dram_tensor("wihd", [KH * 128, NIH * 128], e4,
                           kind="ExternalInput").ap()
    # whh planes: row (m2*128+p) holds NHH plane-items (two k-halves)
    whh_d = nc.dram_tensor("whhd", [KH * 128, NHH * 128], e4,
                           kind="ExternalInput").ap()
    # who planes padded to 16 cols: [kt*128, dig*16] (e5m2 base-16)
    who_d = nc.dram_tensor("whod", [KH * 128, ND6 * 16], e5,
                           kind="ExternalInput").ap()
    cs_hh_d = nc.dram_tensor("cs_hh", [128, KH], f32, kind="ExternalInput").ap()
    cs_ho_d = nc.dram_tensor("cs_ho", [OUTPUT_DIM, 1], f32,
                             kind="ExternalInput").ap()
    id_d = nc.dram_tensor("ident", [128, 128], f32, kind="ExternalInput").ap()
    out_d = nc.dram_tensor("out", [OUTPUT_DIM, BLOC], f32,
                           kind="ExternalOutput").ap()

    with tile.TileContext(nc) as tc, ExitStack() as ctx:
        # spike complements {0, 2^-15}, resident across both phases
        # [p, kt, col]: cols 0..3200 hold u1; 6 ring slots of 640 also serve
        # as u2 staging (a slot is reused once mm2 has consumed its u1 cols)
        u1_pool = ctx.enter_context(tc.tile_pool(name="u1", bufs=1))
        u1 = u1_pool.tile([128, KH * NSLOT * SCMAX], e5, tag="u1")
        u1_3 = u1[:].rearrange("p (k c) -> p k c", k=KH)

        # hi-scale copies of u1, staged per superblock for the hi-plane
        # pairs (320-col pieces; up to 2+3 pieces in flight)
        uhi_pool = ctx.enter_context(tc.tile_pool(name="uhi", bufs=5))

        UHW = SCMAX // 2

        def stage_uhi(c0, cn):
            """Stage {0,1} copies of u1 cols [c0, c0+cn) as 320-col half
            tiles (halves the pool footprint vs full-superblock tiles)."""
            halves = []
            for hc0 in range(0, cn, UHW):
                w = min(UHW, cn - hc0)
                uh = uhi_pool.tile([128, KH * UHW], e5, tag="uhi",
                                   name=f"uhi_{c0 + hc0}")
                uh_3 = uh[:].rearrange("p (k c) -> p k c", k=KH)
                nc.vector.tensor_scalar(uh_3[:, :, 0:w],
                                        u1_3[:, :, c0 + hc0:c0 + hc0 + w],
                                        UHI_VAL, None, op.mult)
                halves.append(uh_3)
            return halves

        # first whh half-chunk (k 0-7 of m2=0), prefetched into space
        # disjoint from phase A's pools so its DMA runs during phase A
        wpre_pool = ctx.enter_context(tc.tile_pool(name="wpre", bufs=1))
        wst_pre = wpre_pool.tile([128, (NHH // 2) * 128], e4, tag="wpre")

        def emit_planes(ps, w_3, kc2, h, mov_lo, mov_hi, c0, cn, first, last,
                        c0h=None, parts="all"):
            """One half's 2.5*kc2 DoubleRow matmuls into PSUM group `ps`.
            w_3: stationary tile viewed [p, item, 128] (items for half h at
            item offset h*5*kc2). mov_lo/mov_hi: [p, k, c] moving tiles;
            c0h: column origin within mov_hi (defaults to c0). parts
            selects the lo-moving pairs ("ac"), hi-moving pairs ("b"), or
            everything ("all") so callers can front-load the work that only
            needs mov_lo while mov_hi's DMA is still in flight."""
            if c0h is None:
                c0h = c0
            base = h * 5 * kc2
            k0 = h * kc2
            idx = 0
            if parts in ("all", "ac"):
                for k in range(k0, k0 + kc2):    # A-pairs (planes 0,1)
                    nc.tensor.matmul(
                        ps, w_3[:, base + idx:base + idx + 2, :],
                        mov_lo[:, k, c0:c0 + cn].unsqueeze(1)
                        .broadcast_to([128, 2, cn]),
                        start=(first and idx == 0), stop=False, perf_mode=DR)
                    idx += 2
                for kk in range(kc2 // 2):       # C-pairs (plane 2, k/k+1)
                    nc.tensor.matmul(
                        ps, w_3[:, base + idx:base + idx + 2, :],
                        mov_lo[:, k0 + 2 * kk:k0 + 2 * kk + 2, c0:c0 + cn],
                        start=False, stop=False, perf_mode=DR)
                    idx += 2
            else:
                idx = 3 * kc2
            if parts in ("all", "b"):
                for k in range(k0, k0 + kc2):    # B-pairs (planes 3,4)
                    nc.tensor.matmul(
                        ps, w_3[:, base + idx:base + idx + 2, :],
                        mov_hi[:, k, c0h:c0h + cn].unsqueeze(1)
                        .broadcast_to([128, 2, cn]),
                        start=False, stop=(last and idx == 5 * kc2 - 2),
                        perf_mode=DR)
                    idx += 2

        def emit_planes4(ps, w_3, kc2, h, mov_lo, mov_hi, c0, cn, first,
                         last, c0h):
            """4-plane variant: 2*kc2 DoubleRow matmuls (A-pairs vs mov_lo,
            B-pairs vs mov_hi), items at offset h*4*kc2."""
            base = h * 4 * kc2
            k0 = h * kc2
            idx = 0
            for k in range(k0, k0 + kc2):        # A-pairs (planes 0,1)
                nc.tensor.matmul(
                    ps, w_3[:, base + idx:base + idx + 2, :],
                    mov_lo[:, k, c0:c0 + cn].unsqueeze(1)
                    .broadcast_to([128, 2, cn]),
                    start=(first and idx == 0), stop=False, perf_mode=DR)
                idx += 2
            for k in range(k0, k0 + kc2):        # B-pairs (planes 2,3)
                nc.tensor.matmul(
                    ps, w_3[:, base + idx:base + idx + 2, :],
                    mov_hi[:, k, c0h:c0h + cn].unsqueeze(1)
                    .broadcast_to([128, 2, cn]),
                    start=False, stop=(last and idx == 4 * kc2 - 2),
                    perf_mode=DR)
                idx += 2

        # ---------------- Phase A: mm1 + LIF1 scan ----------------
        with tc.tile_pool(name="wih", bufs=1) as wih_pool, \
             tc.tile_pool(name="xin", bufs=2) as x_pool, \
             tc.tile_pool(name="cur1", bufs=3) as cur1_pool, \
             tc.tile_pool(name="st1", bufs=1) as st1_pool, \
             tc.tile_pool(name="psA", bufs=6, space="PSUM") as psA:

            def load_x(xt, c0, ncols):
                # one DMA covers both scales (lo then hi in the free dim)
                xt_4 = xt[:].rearrange("p (s k c) -> p s k c", s=2, k=KI)
                nc.sync.dma_start(
                    xt_4,
                    x_d[:, c0:c0 + ncols].rearrange("(s k p) c -> p s k c",
                                                    p=128, s=2))
                return xt_4[:, 0], xt_4[:, 1]

            # wih digit planes, per m-chunk (progressive: mm1 m-chunk can
            # start as soon as its planes land); the m=0 chunk issues first,
            # then the two head x blocks, then the rest of the chunks
            def load_wih(m):
                w = wih_pool.tile([128, NIH * 128], e4, tag=f"wih_{m}")
                nc.sync.dma_start(w[:], wih_d[m * 128:(m + 1) * 128, :])
                return w

            # first wih half-load, then head-x lo, then the second half,
            # then head-x hi: the first A-pair matmuls start after just two
            # small DMAs instead of the whole m=0 chunk + both x tensors
            w0 = wih_pool.tile([128, NIH * 128], e4, tag="wih_0")
            halfb = (NIH // 2) * 128
            nc.sync.dma_start(w0[:, 0:halfb], wih_d[0:128, 0:halfb])
            x_head = x_pool.tile([128, 2 * KI * 3 * CA], e5, tag="xh",
                                 bufs=1)
            xh_4 = x_head[:].rearrange("p (s k c) -> p s k c", s=2, k=KI)
            nc.sync.dma_start(
                xh_4[:, 0],
                xlo_d[:, 0:3 * CA].rearrange("(k p) c -> p k c", p=128))
            nc.sync.dma_start(w0[:, halfb:], wih_d[0:128, halfb:])
            nc.sync.dma_start(
                xh_4[:, 1],
                xhi_d[:, 0:3 * CA].rearrange("(k p) c -> p k c", p=128))
            xhl_3, xhh_3 = xh_4[:, 0], xh_4[:, 1]
            wih_sb = [w0]
            for m in range(1, KH):
                wih_sb.append(load_wih(m))

            y1 = st1_pool.tile([128, KH * BLOC], f32, tag="y1")
            nc.vector.memset(y1[:], 0.0)
            y1_3 = y1[:].rearrange("p (m b) -> p m b", m=KH)

            def mm1_block(xl_3, xh_3, c0, cur1, m):
                ps = psA.tile([128, CA], f32, tag="psA")
                w_3 = wih_sb[m][:].rearrange("p (i f) -> p i f", i=NIH)
                # lo-moving pairs of both halves first: the x_hi DMA (the
                # later half of each x load) is never on the critical path
                for h in range(2):
                    emit_planes(ps[:], w_3, KI // 2, h, xl_3, xh_3,
                                c0, CA, first=(h == 0), last=False,
                                parts="ac")
                for h in range(2):
                    emit_planes(ps[:], w_3, KI // 2, h, xl_3, xh_3,
                                c0, CA, first=False, last=(h == 1),
                                parts="b")
                nc.scalar.copy(cur1[:, m * CA:(m + 1) * CA], ps[:])

            def scan1_block(cur1, c0, ml=0, mh=KH):
                """LIF1 scan; the per-m chains are independent, so callers
                may split the m range to start scanning mid-mm1."""
                cur1_r = cur1[:].rearrange("p (m c) -> p m c", m=KH)
                for t in range(TB):
                    # v computed in-place over the cur1 slice
                    v = cur1_r[:, ml:mh, t * BLOC:(t + 1) * BLOC]
                    ub = u1_3[:, ml:mh,
                              c0 + t * BLOC:c0 + (t + 1) * BLOC]
                    # v = 0.9*y + cur
                    nc.vector.scalar_tensor_tensor(v, y1_3[:, ml:mh, :],
                                                   DECAY, v,
                                                   op.mult, op.add)
                    # u = (v < 1) * 2^-15, e5m2 for the DoubleRow matmul
                    nc.vector.tensor_scalar(ub, v, TH, MOV,
                                            op.is_lt, op.mult)
                    # y = (v<1)*v
                    nc.vector.scalar_tensor_tensor(y1_3[:, ml:mh, :], v,
                                                   TH, v,
                                                   op.is_lt, op.mult)

            # blocks 0-2 interleaved per m-chunk: mm1 rides the progressive
            # wih DMA (3 blocks of PE work per m-chunk outpaces each chunk's
            # DMA, so the PE never starves on the stream); the m 0-7 half of
            # each head scan is emitted mid-stream so the head cur1 buffers
            # free up quickly for the steady blocks
            NHEAD = 3
            cur1_hd = [cur1_pool.tile([128, KH * CA], f32, tag="cur1",
                                      name=f"cur1_hd{b}")
                       for b in range(NHEAD)]
            for m in range(KH):
                for b in range(NHEAD):
                    mm1_block(xhl_3, xhh_3, b * CA, cur1_hd[b], m)
                if m == 8:
                    for b in range(NHEAD):
                        scan1_block(cur1_hd[b], b * CA, 0, 8)
            for b in range(NHEAD):
                scan1_block(cur1_hd[b], b * CA, 8, KH)

            uhi_ready = {}
            for blk in range(NHEAD, NBLK):
                c0 = blk * CA
                xt = x_pool.tile([128, 2 * KI * CA], e5, tag="x")
                xl_3, xh_3 = load_x(xt, c0, CA)
                if blk == NHEAD + 2:
                    # prefetch the first whh half-chunk once the head x
                    # loads are through: it transfers during phase A, ready
                    # long before the phase seam
                    nc.sync.dma_start(wst_pre[:],
                                      whh_d[0:128, 0:(NHH // 2) * 128])
                cur1 = cur1_pool.tile([128, KH * CA], f32, tag="cur1")
                for m in range(KH):
                    mm1_block(xl_3, xh_3, 0, cur1, m)
                scan1_block(cur1, c0)
                # stage the {0,1} copies for superblocks 0 and 1 as soon as
                # their u1 columns exist, so the phase seam has no DVE stall
                if blk == 3:
                    uhi_ready[0] = stage_uhi(*SUPERS[0])
                if blk == 7:
                    uhi_ready[1] = stage_uhi(*SUPERS[1])

        # ---------------- Phase B: mm2 + LIF2 + mm3 + output scan -----------
        with tc.tile_pool(name="wst", bufs=3) as wst_pool, \
             tc.tile_pool(name="cur2", bufs=2) as cur2_pool, \
             tc.tile_pool(name="smallB", bufs=1) as sm_pool, \
             tc.tile_pool(name="cur3", bufs=1) as cur3_pool, \
             tc.tile_pool(name="s3p", bufs=2) as s3_pool, \
             tc.tile_pool(name="psB", bufs=4, space="PSUM") as psB, \
             tc.tile_pool(name="ps3", bufs=2, space="PSUM") as ps3_pool, \
             tc.tile_pool(name="pstr", bufs=2, space="PSUM") as pstr_pool:

            who_sb = sm_pool.tile([128, KH * ND6 * 16], e5, tag="who")
            who4 = who_sb[:].rearrange("p (k i m) -> p k i m", k=KH, i=ND6)
            cs_hh = sm_pool.tile([128, KH], f32, tag="cshh")
            cs_ho = sm_pool.tile([OUTPUT_DIM, 1], f32, tag="csho")
            ident_sb = sm_pool.tile([128, 128], f32, tag="ident")

            nc.sync.dma_start(cs_hh[:], cs_hh_d[:, :])
            nc.sync.dma_start(
                who_sb[:].rearrange("p (k f) -> p k f", k=KH),
                who_d[:, :].rearrange("(k p) f -> p k f", p=128))
            nc.sync.dma_start(cs_ho[:], cs_ho_d[:, :])
            nc.sync.dma_start(ident_sb[:], id_d[:, :])

            y2 = sm_pool.tile([128, KH * BLOC], f32, tag="y2")
            yo = sm_pool.tile([OUTPUT_DIM, BLOC], f32, tag="yo")
            vo = sm_pool.tile([OUTPUT_DIM, BLOC], f32, tag="vo")
            acc0 = sm_pool.tile([OUTPUT_DIM, BLOC], f32, tag="acc0")
            acc1 = sm_pool.tile([OUTPUT_DIM, BLOC], f32, tag="acc1")
            acc = [acc0, acc1]
            out_sb = sm_pool.tile([OUTPUT_DIM, BLOC], f32, tag="rate")
            nc.vector.memset(y2[:], 0.0)
            nc.vector.memset(yo[:], 0.0)
            nc.vector.memset(acc[0][:], 0.0)
            y2_3 = y2[:].rearrange("p (m b) -> p m b", m=KH)

            def emit_mm3(c0, uoff, cn):
                """mm3 (transposed: moving side = W_ho planes, 10-wide
                output) + PE transpose back + output-layer scan."""
                cur3 = cur3_pool.tile([OUTPUT_DIM, 768], f32, tag="cur3")
                for ch in range(cn // 128):
                    ps3 = ps3_pool.tile([128, OUTPUT_DIM], f32, tag="ps3")
                    for k in range(KH // 2):
                        # stationary: u2 k-tile pair; moving: W_ho planes
                        ub = u1_3[:, 2 * k:2 * k + 2,
                                  uoff + ch * 128:uoff + (ch + 1) * 128]
                        for i in range(ND6):
                            nc.tensor.matmul(
                                ps3[:],
                                ub,
                                who4[:, 2 * k:2 * k + 2, i,
                                     0:OUTPUT_DIM],
                                start=(k == 0 and i == 0),
                                stop=(k == KH // 2 - 1 and i == ND6 - 1),
                                perf_mode=DR)
                    s3 = s3_pool.tile([128, OUTPUT_DIM], f32, tag="s3")
                    nc.scalar.copy(s3[:], ps3[:])
                    pst = pstr_pool.tile([OUTPUT_DIM, 128], f32, tag="pst")
                    nc.tensor.transpose(pst[:], s3[:], ident_sb[:])
                    # cur3 = colsum_ho - u2@W_ho  (true output current)
                    nc.scalar.activation(cur3[:, ch * 128:(ch + 1) * 128],
                                         pst[:], ident,
                                         bias=cs_ho[:, 0:1], scale=-1.0)
                # output-layer scan runs on the otherwise-idle GPSIMD
                # engine so the tail never serializes behind the DVE scan
                for t in range(cn // BLOC):
                    g = c0 // BLOC + t
                    sl = cur3[:, t * BLOC:(t + 1) * BLOC]
                    nc.vector.scalar_tensor_tensor(vo[:], yo[:], DECAY, sl,
                                                   op.mult, op.add)
                    nc.vector.scalar_tensor_tensor(acc[(g + 1) % 2][:], vo[:],
                                                   TH, acc[g % 2][:],
                                                   op.is_lt, op.add)
                    nc.vector.scalar_tensor_tensor(yo[:], vo[:], TH, vo[:],
                                                   op.is_lt, op.mult)

            prev = None
            for sup, (c0, cn) in enumerate(SUPERS):
                # u2(s) overwrites u1's own columns: by scan time mm2(s)
                # has consumed every u1 column of this superblock
                uoff = c0
                uhalves = uhi_ready.pop(sup)
                # chunk the column range so each PSUM tile fits one bank
                # and each chunk sits inside one 320-col uhi staging piece
                chunks = [(o, min(UHW, cn - o)) for o in range(0, cn, UHW)]
                # alternate tags so the two live cur2 buffers are sized
                # for the supers that actually use them (768 + 640 fits;
                # a single max-sized tag x2 bufs would not)
                cur2 = cur2_pool.tile([128, KH * cn], f32,
                                      tag=f"cur2{sup % 2}", bufs=1,
                                      name=f"cur2_{sup}")
                cur2_r = cur2[:].rearrange("p (m c) -> p m c", m=KH)
                for m2 in range(KH):
                    if m2 == 8 and prev is not None:
                        emit_mm3(*prev)
                        prev = None
                    if m2 == 8 and sup + 2 < len(SUPERS):
                        # stage the {0,1} copy two superblocks ahead
                        uhi_ready[sup + 2] = stage_uhi(*SUPERS[sup + 2])
                    wst = wst_pool.tile([128, NHH * 128], e4, tag="wst")
                    if sup == 0 and m2 == 0:
                        # k 8-15 only; k 0-7 comes from the prefetched half
                        nc.sync.dma_start(
                            wst[:, (NHH // 2) * 128:],
                            whh_d[0:128, (NHH // 2) * 128:])
                    else:
                        nc.sync.dma_start(
                            wst[:], whh_d[m2 * 128:(m2 + 1) * 128, :])
                    wst_3 = wst[:].rearrange("p (i f) -> p i f", i=NHH)
                    wpre_3 = wst_pre[:].rearrange("p (i f) -> p i f",
                                                  i=NHH // 2)
                    for (off, ncol) in chunks:
                        ps = psB.tile([128, ncol], f32, tag="psB")
                        uh_3 = uhalves[off // UHW]
                        for h in range(2):
                            use_pre = (sup == 0 and m2 == 0 and h == 0)
                            w_3 = wpre_3 if use_pre else wst_3
                            # the prefetch tile holds half 0 at offset 0
                            hh = 0 if use_pre else h
                            emit_planes4(ps[:], w_3, KH // 2, hh, u1_3, uh_3,
                                         c0 + off, ncol,
                                         first=(h == 0), last=(h == 1),
                                         c0h=off % UHW)
                        # cur2 = colsum_hh - u1@W_hh (true layer-2 current)
                        nc.scalar.activation(
                            cur2_r[:, m2, off:off + ncol],
                            ps[:], ident, bias=cs_hh[:, m2:m2 + 1],
                            scale=-1.0)
                for t in range(cn // BLOC):
                    # v computed in-place over the cur2 slice
                    v = cur2_r[:, :, t * BLOC:(t + 1) * BLOC]
                    ub = u1_3[:, :,
                              uoff + t * BLOC:uoff + (t + 1) * BLOC]
                    nc.vector.scalar_tensor_tensor(v, y2_3, DECAY, v,
                                                   op.mult, op.add)
                    nc.vector.tensor_scalar(ub, v, TH, MOV,
                                            op.is_lt, op.mult)
                    nc.vector.scalar_tensor_tensor(y2_3, v, TH, v,
                                                   op.is_lt, op.mult)
                prev = (c0, uoff, cn)
            emit_mm3(*prev)

            # rate = 1 - acc/T   (acc holds sum of u_out; s = 1-u)
            nc.vector.tensor_scalar(out_sb[:], acc[T % 2][:], -1.0 / T, 1.0,
                                    op.mult, op.add)
            nc.sync.dma_start(out_d[:, :], out_sb[:])

    nc.compile()
    return nc


def _digit_planes32(w, nplanes, kbits, pscale):
    """Decompose fp32 weights into nplanes exact e4m3 base-32 digit planes.

    w ~= Wfix * 2^-kbits with Wfix = sum_i d_i 32^i, d_i in [-16,15].
    Plane i holds d_i * pscale[i]; the moving operand supplies the rest of
    each plane's 32^i * 2^-kbits scale so every product is fp32-exact.
    Returns (planes [nplanes, *w.shape] e4m3-exact fp32, effective weights
    fp32)."""
    dmax = 16 * (32 ** nplanes - 1) // 31
    wfix = np.round(w.astype(np.float64) * (1 << kbits)).astype(np.int64)
    assert np.abs(wfix).max() <= dmax, "weights exceed digit range"
    planes = np.zeros((nplanes,) + w.shape, np.float32)
    rem = wfix.copy()
    for i in range(nplanes):
        d = ((rem + 16) % 32) - 16
        rem = (rem - d) >> 5
        planes[i] = d * np.float32(pscale[i])
    assert np.all(rem == 0)
    weff = (wfix * (2.0 ** -kbits)).astype(np.float32)
    return planes, weff


def _digit_planes16(w):
    """Base-16 e5m2 planes for W_ho (moving side of mm3; u2 carries 2^-15).

    w ~= Wfix * 2^-KB23, plane i holds d_i * 2^(4i - KB23 + 15)."""
    wfix = np.round(w.astype(np.float64) * (1 << KB23)).astype(np.int64)
    assert np.abs(wfix).max() <= DMAX16, "weights exceed digit range"
    planes = np.zeros((ND6,) + w.shape, np.float32)
    rem = wfix.copy()
    for i in range(ND6):
        d = ((rem + 8) % 16) - 8
        rem = (rem - d) >> 4
        planes[i] = d * np.float32(2.0 ** (4 * i - KB23 + 15))
    assert np.all(rem == 0)
    weff = (wfix * (2.0 ** -KB23)).astype(np.float32)
    return planes, weff


def _pack_planes(planes, kc, mc, items_fn):
    """Pack [nplanes, K, M] planes into the DMA layout: row (m*128+p) holds
    the nplanes*kc plane-items (two k-halves, DR-pair-adjacent) of 128
    bytes each."""
    nitem = len(planes) * kc
    out = np.zeros((mc, 128, nitem, 128), np.float32)
    for m in range(mc):
        mcols = slice(m * 128, (m + 1) * 128)
        idx = 0
        for h in range(2):
            for (k, pl) in items_fn(kc // 2, h):
                out[m, :, idx, :] = planes[pl][k * 128:(k + 1) * 128, mcols]
                idx += 1
    return out.reshape(mc * 128, nitem * 128)


def kernel(input_bins, W_ih, W_hh, W_ho):
    global _BUILT
    if _BUILT is None:
        _BUILT = _build()
    nc = _BUILT
    import ml_dtypes
    e5np = ml_dtypes.float8_e5m2
    e4np = ml_dtypes.float8_e4m3

    input_bins = np.ascontiguousarray(input_bins, dtype=np.float32)
    W_ih = np.ascontiguousarray(W_ih, dtype=np.float32)
    W_hh2 = np.ascontiguousarray(np.asarray(W_hh)[0], dtype=np.float32)
    W_ho = np.ascontiguousarray(W_ho, dtype=np.float32)

    pih, wih_eff = _digit_planes32(W_ih, NP_IH, KBITS_IH, PSCALE_IH)
    phh, whh_eff = _digit_planes32(W_hh2, NP_HH, KBITS_HH, PSCALE_HH)
    pho, who_eff = _digit_planes16(W_ho)     # [ND6, 2048, 10]

    wihd = np.ascontiguousarray(
        _pack_planes(pih, KI, KH, _half_items5)).astype(e4np)
    whhd = np.ascontiguousarray(
        _pack_planes(phh, KH, KH, _half_items4)).astype(e4np)

    # who planes padded to 16 output cols: [kt*128, dig*16]
    whod = np.zeros((KH, 128, ND6, 16), np.float32)
    whod[:, :, :, :OUTPUT_DIM] = pho.reshape(ND6, KH, 128, OUTPUT_DIM) \
        .transpose(1, 2, 0, 3)
    whod8 = np.ascontiguousarray(whod.reshape(KH * 128, ND6 * 16)).astype(e5np)

    cs_hh = np.ascontiguousarray(
        whh_eff.sum(axis=0, dtype=np.float64).astype(np.float32)
        .reshape(KH, 128).T)
    cs_ho = who_eff.sum(axis=0, dtype=np.float64).astype(np.float32) \
        .reshape(OUTPUT_DIM, 1)

    in_maps = []
    for c in range(NCORES):
        xb = input_bins[c * BLOC:(c + 1) * BLOC]        # [32, 1024, 100]
        xt = xb.transpose(1, 2, 0).reshape(INPUT_DIM, COLS)
        xall = np.empty((2 * INPUT_DIM, COLS), e5np)
        xall[0:INPUT_DIM] = (xt * np.float32(MOV)).astype(e5np)
        xall[INPUT_DIM:] = xt.astype(e5np)
        in_maps.append({
            "xall": np.ascontiguousarray(xall), "wihd": wihd, "whhd": whhd,
            "whod": whod8, "cs_hh": cs_hh, "cs_ho": cs_ho,
            "ident": np.eye(128, dtype=np.float32),
        })

    from concourse.bass_utils import run_bass_kernel_spmd
    res = run_bass_kernel_spmd(nc, in_maps, core_ids=list(range(NCORES)))

    out = np.empty((BATCH, OUTPUT_DIM), dtype=np.float32)
    for c in range(NCORES):
        out[c * BLOC:(c + 1) * BLOC] = res.results[c]["out"].T
    return out
